# revision 58
# baseline (speedup 1.0000x reference)
"""Trainium2 Bass kernel for AGATCellWithMLP (gnn_message_passing).

Data-parallel across 8 NeuronCores: core b owns graph b (B=8).  Hypernetwork
weights are replicated and streamed; no collectives.

Math (per core, NOBS=512 selected rows n, CD=513 features, IS=256 out):
  combined = [x | h]                                  [1024, 513]
  scoresT[j, n] = leaky(src[n] + tgt[j] + ab)          per head
  attnT[j, n]  = exp(scoresT) * (0.5 / colsum)        (softmax + head-mean)
  selT[d, n]   = sum_h (combined^T @ attnT_h)         [513, 512]
  g(W, inT)[o, n] = sum_{d,i} qT[d,n] * inT[i,n] * W[d,i,o] + (b^T q)[o,n]
  r = sigmoid(g(Wr, selT)); u = sigmoid(g(Wu, selT))
  cT = [x_selT | r*h_selT]
  cand = tanh(g(Wc, cT))
  out[o, n] = (1-u)*r*h_selT + u*cand   -> transpose -> [512, 256]

Performance structure (from trace analysis):
 - The PE runs matmuls back-to-back at ~217ns/MM ([128c,128o,512n], LDWEIGHTS
   hidden) when its queue has no unsatisfied waits; the kernel is built so
   every stage keeps producers >=1 iteration ahead of the PE.
 - Attention is computed almost entirely on the PE: C^T via transposes
   (reused later as xs/hs), src/tgt as a [128d x 4] matmul over C^T, and
   scoresT[j,n] = src[n] + tgt[j] as a rank-3 matmul ([tgt_hi; tgt_lo; ones]
   x [ones; ones; src]) so the DVE only does the leaky-relu.
 - r/u contraction runs in fp8 e4m3 DoubleRow.  The per-d z = selT * q_d
   fp8 production (the old bottleneck: fp8 stores cost ~2.4ns/elem on DVE)
   is split: DVE writes chunks 0-1 fp8-direct (merged op ~1.4ns/elem), and
   produces chunks 2-3 in bf16 (~0.73ns/elem) which the otherwise-idle
   scalar engine converts to fp8.
 - q_d broadcast rows are DMA'd in batches of 8 d's (one descriptor set per
   batch instead of per d).

Precision: host pre-scales qT by SQ=8 and the r/u weight set by SW=64 to
center fp8 e4m3 dynamic range; the device unscales inside the sigmoid.  The
candidate gate stays bf16 (tanh preact std ~13; fp8 fails the 2e-2 gate).
"""

import numpy as np
import ml_dtypes

BF16 = ml_dtypes.bfloat16
F8 = ml_dtypes.float8_e4m3

B, N, IS, QD, H = 8, 1024, 256, 128, 2
CD = 2 * IS + 1  # 513
NOBS = N // 2  # 512
K = B * NOBS  # 4096
P = 128
NJ = N // P  # 8 j-chunks
NDC = 4  # full d-chunks of the 513 contraction (tail row handled separately)

SQ = 8.0  # host scale on qT (folded out in the gate activations)
SW = 64.0  # host scale on the r/u weight set (fp8 dynamic-range centering)

# shifted column chunks of `combined` used for C^T / xs / hs (tail col 256
# handled separately so x rows [0:256) and h rows [257:513) stay aligned)
CT_COLS = [(0, 128), (128, 256), (257, 385), (385, 513)]

_CACHE = {}


def _split_waits(nc, mybir):
    # This toolchain's walrus allows only ONE sync-wait per instruction
    # ("Too many sync wait commands"); hoist extra waits onto standalone
    # same-engine EventSemaphore nops placed immediately before.
    n = 0
    for f in nc.m.functions:
        for blk in f.blocks:
            out = []
            for inst in blk.instructions:
                si = inst.sync_info
                if si is not None and si.on_wait and len(si.on_wait) > 1:
                    waits = list(si.on_wait)
                    for w in waits[:-1]:
                        ev = mybir.InstEventSemaphore(
                            name=f"I-wsplit-{n}", engine=inst.engine, ins=[], outs=[]
                        )
                        ev.sync_info = mybir.SyncInfo(on_wait=[w], on_update=[])
                        out.append(ev)
                        n += 1
                    inst.sync_info = mybir.SyncInfo(
                        on_wait=[waits[-1]], on_update=list(si.on_update or [])
                    )
                out.append(inst)
            blk.instructions = out


def _build(ab0: float, ab1: float):
    import concourse.bass as bass
    import concourse.mybir as mybir
    import concourse.tile as tile
    from concourse.masks import make_identity
    from contextlib import ExitStack

    dt = mybir.dt
    AF = mybir.ActivationFunctionType
    ALU = mybir.AluOpType
    DR = mybir.MatmulPerfMode.DoubleRow

    nc = bass.Bass()
    # The kernel-tail EVENT_SEMAPHORE_RANGE_CLEAR (InstISA) is rejected by this
    # toolchain's walrus ("ISA wrong length"); the NEFF runs once per load, so
    # skipping the tail semaphore clear is safe.
    nc.clear_and_free_semaphores = lambda sems: None

    cmb_d = nc.declare_dram_parameter("cmb", [N, CD], dt.bfloat16, isOutput=False)
    qT_d = nc.declare_dram_parameter("qT", [QD, NOBS], dt.bfloat16, isOutput=False)
    w4t_d = nc.declare_dram_parameter("w4t", [P, NDC, 4], dt.bfloat16, isOutput=False)
    w4tail_d = nc.declare_dram_parameter("w4tail", [1, 4], dt.bfloat16, isOutput=False)
    wru_d = nc.declare_dram_parameter(
        "wru", [QD, P, NDC, 2, IS], dt.float8e4, isOutput=False
    )
    wrut_d = nc.declare_dram_parameter("wrut", [QD, 2, IS], dt.bfloat16, isOutput=False)
    wc_d = nc.declare_dram_parameter("wc", [QD, P, NDC, IS], dt.bfloat16, isOutput=False)
    wct_d = nc.declare_dram_parameter("wct", [QD, IS], dt.bfloat16, isOutput=False)
    bru_d = nc.declare_dram_parameter("bru", [QD, 2, IS], dt.bfloat16, isOutput=False)
    bc_d = nc.declare_dram_parameter("bc", [QD, IS], dt.bfloat16, isOutput=False)
    abv_d = nc.declare_dram_parameter("abv", [4, 1], dt.float32, isOutput=False)
    out_d = nc.declare_dram_parameter("out", [NOBS, IS], dt.float32, isOutput=True)
    # DRAM bounce buffers (partition remaps / broadcasts)
    sttail_d = nc.dram_tensor("sttail_dram", [1, NOBS], dt.bfloat16)
    xtail_d = nc.dram_tensor("xtail_dram", [1, NOBS], dt.bfloat16)
    tgthi_d = nc.dram_tensor("tgthi_dram", [H, N], dt.bfloat16)
    tgtlo_d = nc.dram_tensor("tgtlo_dram", [H, N], dt.bfloat16)
    src_d = nc.dram_tensor("src_dram", [H, NOBS], dt.bfloat16)

    with tile.TileContext(nc) as tc, ExitStack() as ctx:
        consts = ctx.enter_context(tc.tile_pool(name="consts", bufs=1))
        sb = ctx.enter_context(tc.tile_pool(name="sb", bufs=1))
        scratch = ctx.enter_context(tc.tile_pool(name="scratch", bufs=2))
        qbpool = ctx.enter_context(tc.tile_pool(name="qbpool", bufs=3))
        e2pool = ctx.enter_context(tc.tile_pool(name="e2pool", bufs=3))
        zpool = ctx.enter_context(tc.tile_pool(name="zpool", bufs=2))
        z4pool = ctx.enter_context(tc.tile_pool(name="z4pool", bufs=3))
        zbpool = ctx.enter_context(tc.tile_pool(name="zbpool", bufs=2))
        wpool = ctx.enter_context(tc.tile_pool(name="wpool", bufs=6))
        wcpool = ctx.enter_context(tc.tile_pool(name="wcpool", bufs=6))
        wcxpool = ctx.enter_context(tc.tile_pool(name="wcxpool", bufs=4))

        # ---------------- constants ----------------
        id_bf = consts.tile([P, P], dt.bfloat16)
        make_identity(nc, id_bf)
        id_f32 = consts.tile([P, P], dt.float32)
        make_identity(nc, id_f32)
        ones_col = consts.tile([P, 1], dt.bfloat16)
        nc.gpsimd.memset(ones_col, 1.0)
        ones_row = consts.tile([1, P], dt.bfloat16)
        nc.gpsimd.memset(ones_row, 1.0)

        # ---------------- inputs to SBUF ----------------
        C = sb.tile([P, NJ, CD], dt.bfloat16)  # combined, j-chunked
        # per-jc DMAs so the first transposes start as soon as chunk 0 lands
        for jc in range(NJ):
            nc.sync.dma_start(C[:, jc, :], cmb_d[jc * P : (jc + 1) * P, :])
        qT = sb.tile([P, NOBS], dt.bfloat16)
        nc.sync.dma_start(qT[:], qT_d[:])
        w4t = sb.tile([P, NDC, 4], dt.bfloat16)
        nc.sync.dma_start(w4t[:], w4t_d[:])
        w4tail = sb.tile([1, 4], dt.bfloat16)
        nc.sync.dma_start(w4tail[:], w4tail_d[:])
        wrut = sb.tile([P, 2, IS], dt.bfloat16)
        nc.sync.dma_start(wrut[:], wrut_d[:])
        wct = sb.tile([P, IS], dt.bfloat16)
        nc.sync.dma_start(wct[:], wct_d[:])
        bru = sb.tile([P, 2, IS], dt.bfloat16)
        nc.sync.dma_start(bru[:], bru_d[:])
        bc = sb.tile([P, IS], dt.bfloat16)
        nc.sync.dma_start(bc[:], bc_d[:])
        abv = sb.tile([4, 1], dt.float32)
        nc.sync.dma_start(abv[:], abv_d[:])

        # ---------------- c-gate accumulators (opened early: the x-part
        # matmuls are independent of attention and prepaid into its PE idle)
        psE_cm = tc.tile_pool(name="psE", bufs=1, space="PSUM")
        psE = psE_cm.__enter__()
        cacc = [
            psE.tile([P, NOBS], dt.float32, tag=f"cacc{oc}", name=f"cacc{oc}")
            for oc in range(2)
        ]
        for oc in range(2):
            nc.tensor.matmul(
                cacc[oc][:], bc[:, oc * P : (oc + 1) * P], qT[:], start=True, stop=False
            )

        def qb_batch(t8):
            qbt = qbpool.tile([P, 8, NOBS], dt.bfloat16, tag="qb8")
            nc.sync.dma_start(
                qbt[:],
                qT_d[t8 * 8 : t8 * 8 + 8, :]
                .rearrange("(o a) b -> o a b", o=1)
                .to_broadcast([P, 8, NOBS]),
            )
            return qbt

        # ---------------- C^T (all j), tails ----------------
        CT = sb.tile([P, NDC, NJ, P], dt.bfloat16)  # shifted chunks, see CT_COLS
        ctail = sb.tile([1, NJ, P], dt.bfloat16)  # combined col 256 per jc
        xsv = CT[:, 0:2, 0:4, :].rearrange("p (o a) jc b -> p o a jc b", o=1)

        PRE = 20  # c-gate x-part d-pairs prepaid during attention
        _pre = {"t": 0, "qb": None}

        def prepay_pair():
            t = _pre["t"]
            if t >= PRE:
                return
            _pre["t"] += 1
            dm = (2 * t) % 8
            if dm == 0:
                _pre["qb"] = qb_batch(t // 4)
            qbx = _pre["qb"]
            wcxs = []
            for dp in range(2):
                w = wcxpool.tile([P, 2, IS], dt.bfloat16, tag="wcx")
                nc.sync.dma_start(w[:], wc_d[2 * t + dp][:, 0:2, :])
                wcxs.append(w)
            qbp = qbx[:, dm : dm + 2, :].rearrange("p a (o b) -> p a o b", o=1)
            xq = zpool.tile([P, 2, NDC, NOBS], dt.bfloat16, tag="z2")
            nc.vector.tensor_mul(
                xq[:, :, 0:2, :],
                xsv.to_broadcast([P, 2, 2, NJ // 2, P]),
                qbp.to_broadcast([P, 2, 2, NOBS]),
            )
            for dp in range(2):
                for ic in range(2):
                    for oc in range(2):
                        nc.tensor.matmul(
                            cacc[oc][:],
                            wcxs[dp][:, ic, oc * P : (oc + 1) * P],
                            xq[:, dp, ic, :],
                            start=False,
                            stop=False,
                        )

        psT_cm = tc.tile_pool(name="psT", bufs=2, space="PSUM")
        psT = psT_cm.__enter__()

        def transpose_jc(jc):
            tp = psT.tile([P, NDC, P], dt.bfloat16, tag="tp")
            for c, (lo, hi) in enumerate(CT_COLS):
                nc.tensor.transpose(tp[:, c, :], C[:, jc, lo:hi], id_bf)
            # alternate ACT/DVE so the copy chain isn't serial on one engine
            if jc % 2 == 0:
                nc.scalar.copy(CT[:, :, jc, :], tp[:])
            else:
                nc.vector.tensor_copy(CT[:, :, jc, :], tp[:])
            tq = psT.tile([1, P], dt.bfloat16, tag="tq")
            nc.tensor.transpose(tq[:], C[:, jc, 256:257], id_bf)
            nc.scalar.copy(ctail[:, jc, :], tq[:])

        for jc in range(4):
            transpose_jc(jc)
        prepay_pair()  # xs (CT jc 0-3, chunks 0-1) is ready now
        prepay_pair()
        for jc in range(4, NJ):
            transpose_jc(jc)

        # ---------------- src/tgt via PE ----------------
        # A[k, j] for k in (tgt0, tgt1, src0, src1); two j-halves of 512
        As = sb.tile([4, 2, NOBS], dt.bfloat16)
        Alo = sb.tile([2, 2, NOBS], dt.bfloat16)
        psA_cm = tc.tile_pool(name="psA", bufs=2, space="PSUM")
        psA = psA_cm.__enter__()
        for half in range(2):
            A = psA.tile([4, NOBS], dt.float32, tag="A")
            for c in range(NDC):
                nc.tensor.matmul(
                    A[:],
                    w4t[:, c, :],
                    CT[:, c, half * 4 : half * 4 + 4, :],
                    start=(c == 0),
                    stop=False,
                )
            nc.tensor.matmul(
                A[:],
                w4tail[:],
                ctail[:, half * 4 : half * 4 + 4, :],
                start=False,
                stop=True,
            )
            # cast (+attn_b on the src rows); keep a bf16-lo residual for tgt
            nc.scalar.activation(As[:, half, :], A[:], AF.Identity, bias=abv[:])
            nc.vector.tensor_sub(Alo[:, half, :], A[0:2, :], As[0:2, half, :])

        # partition remap via DRAM bounce: TT = [tgt_hi; tgt_lo; ones],
        # RR = [ones; ones; src+ab]
        TT = sb.tile([3, H, NJ, P], dt.bfloat16)
        RR = sb.tile([3, H, NOBS], dt.bfloat16)
        # memset whole tiles (partition base must be 0); DMAs below overwrite
        # the non-ones rows
        nc.gpsimd.memset(TT[:], 1.0)
        nc.gpsimd.memset(RR[:], 1.0)
        for hh in range(H):
            for half in range(2):
                nc.sync.dma_start(
                    tgthi_d[hh : hh + 1, half * NOBS : (half + 1) * NOBS],
                    As[hh : hh + 1, half, :],
                )
                nc.sync.dma_start(
                    tgtlo_d[hh : hh + 1, half * NOBS : (half + 1) * NOBS],
                    Alo[hh : hh + 1, half, :],
                )
            nc.sync.dma_start(src_d[hh : hh + 1, :], As[2 + hh : 3 + hh, 0, :])
        for hh in range(H):
            nc.sync.dma_start(
                TT[0:1, hh, :, :], tgthi_d[hh : hh + 1, :].rearrange("o (jc p) -> o jc p", p=P)
            )
            nc.sync.dma_start(
                TT[1:2, hh, :, :], tgtlo_d[hh : hh + 1, :].rearrange("o (jc p) -> o jc p", p=P)
            )
            nc.sync.dma_start(RR[2:3, hh, :], src_d[hh : hh + 1, :])
        psA_cm.__exit__(None, None, None)
        psT_cm.__exit__(None, None, None)
        prepay_pair()
        prepay_pair()

        # ---------------- scores -> exp (+ column sums) ----------------
        expT = sb.tile([P, H, NJ, NOBS], dt.bfloat16)
        psZ_cm = tc.tile_pool(name="psZ", bufs=1, space="PSUM")
        psZ = psZ_cm.__enter__()
        psS_cm = tc.tile_pool(name="psS", bufs=4, space="PSUM")
        psS = psS_cm.__enter__()
        zt = []
        for hh in range(H):
            zth = psZ.tile([1, NOBS], dt.float32, tag=f"zt{hh}", name=f"zt{hh}")
            zt.append(zth)
            for jc in range(NJ):
                sc = psS.tile([P, NOBS], dt.float32, tag="sc")
                nc.tensor.matmul(
                    sc[:], TT[:, hh, jc, :], RR[:, hh, :], start=True, stop=True
                )
                # exp(leaky(x)) == max(exp(x), exp(0.2x)) — avoids a PSUM-read
                # TensorScalarPtr (BIR verifier rejects it) and ACT table swaps
                nc.scalar.activation(expT[:, hh, jc, :], sc[:], AF.Exp)
                e2 = e2pool.tile([P, NOBS], dt.bfloat16, tag="e2")
                nc.scalar.activation(e2[:], sc[:], AF.Exp, scale=0.2)
                nc.vector.tensor_max(expT[:, hh, jc, :], expT[:, hh, jc, :], e2[:])
                nc.tensor.matmul(
                    zth[:],
                    ones_col[:],
                    expT[:, hh, jc, :],
                    start=(jc == 0),
                    stop=(jc == NJ - 1),
                )
                prepay_pair()
        psS_cm.__exit__(None, None, None)

        # ---------------- invZ = 0.5/colsum, broadcast to [128, n] ----------
        # reciprocal on a [128, 4] transposed layout (DVE recip is per-lane
        # serial: [1,512] costs ~4us, [128,4] is ~free)
        psI_cm = tc.tile_pool(name="psI", bufs=1, space="PSUM")
        psI = psI_cm.__enter__()
        invZB = sb.tile([P, H, NOBS], dt.bfloat16)
        for hh in range(H):
            ztsb = scratch.tile([1, NOBS], dt.float32, tag="ztsb")
            nc.scalar.activation(ztsb[:], zt[hh][:], AF.Copy, scale=2.0)
            ztc = psI.tile([P, NDC], dt.float32, tag="ztc")
            for c in range(NDC):
                nc.tensor.transpose(
                    ztc[:, c : c + 1], ztsb[:, c * P : (c + 1) * P], id_f32[0:1, 0:1]
                )
            ztcs = scratch.tile([P, NDC], dt.float32, tag="ztcs")
            nc.scalar.copy(ztcs[:], ztc[:])
            izc = scratch.tile([P, NDC], dt.float32, tag="izc")
            nc.vector.reciprocal(izc[:], ztcs[:])
            izr = psI.tile([1, NOBS], dt.float32, tag="izr")
            for c in range(NDC):
                nc.tensor.transpose(izr[:, c * P : (c + 1) * P], izc[:, c : c + 1], id_f32)
            izrb = scratch.tile([1, NOBS], dt.bfloat16, tag="izrb")
            nc.scalar.copy(izrb[:], izr[:])
            ib = psI.tile([P, NOBS], dt.float32, tag="ib")
            nc.tensor.matmul(ib[:], ones_row[:], izrb[:], start=True, stop=True)
            nc.scalar.copy(invZB[:, hh, :], ib[:])
        psI_cm.__exit__(None, None, None)
        psZ_cm.__exit__(None, None, None)

        # ---------------- attnT = expT * invZB ----------------
        attnT = sb.tile([P, H, NJ, NOBS], dt.bfloat16)
        for hh in range(H):
            nc.vector.tensor_mul(
                attnT[:, hh, :, :],
                expT[:, hh, :, :],
                invZB[:, hh : hh + 1, :].to_broadcast([P, NJ, NOBS]),
            )
            prepay_pair()

        # ---------------- selT = C^T @ attnT (summed over heads) -----------
        selT = sb.tile([P, NDC, NOBS], dt.bfloat16)
        selTtail = sb.tile([1, NOBS], dt.bfloat16)
        psL_cm = tc.tile_pool(name="psL", bufs=1, space="PSUM")
        psL = psL_cm.__enter__()
        selps = []
        for dc in range(NDC):
            sp = psL.tile([P, NOBS], dt.float32, tag=f"sel{dc}", name=f"sel{dc}")
            selps.append(sp)
            first = True
            for hh in range(H):
                for jc in range(NJ):
                    nc.tensor.matmul(
                        sp[:],
                        C[:, jc, dc * P : (dc + 1) * P],
                        attnT[:, hh, jc, :],
                        start=first,
                        stop=(hh == H - 1 and jc == NJ - 1),
                    )
                    first = False
            nc.scalar.copy(selT[:, dc, :], sp[:])
            prepay_pair()
        zq = psL.tile([1, NOBS], dt.float32, tag="zq", name="zq")
        first = True
        for hh in range(H):
            for jc in range(NJ):
                nc.tensor.matmul(
                    zq[:],
                    C[:, jc, 512:513],
                    attnT[:, hh, jc, :],
                    start=first,
                    stop=(hh == H - 1 and jc == NJ - 1),
                )
                first = False
        nc.scalar.copy(selTtail[:], zq[:])
        for _ in range(6):
            prepay_pair()
        psL_cm.__exit__(None, None, None)

        # ---------------- gate phase 1: r and u (fp8 DoubleRow) -------------
        psD_cm = tc.tile_pool(name="psD", bufs=1, space="PSUM")
        psD = psD_cm.__enter__()
        acc = {}
        for g in range(2):
            for oc in range(2):
                acc[(g, oc)] = psD.tile(
                    [P, NOBS], dt.float32, tag=f"acc{g}{oc}", name=f"acc{g}{oc}"
                )
        for g in range(2):
            for oc in range(2):
                nc.tensor.matmul(
                    acc[(g, oc)][:],
                    bru[:, g, oc * P : (oc + 1) * P],
                    qT[:],
                    start=True,
                    stop=False,
                )
        nc.sync.dma_start(sttail_d[:], selTtail[:])

        # d-pair batched z production: one DVE op covers two d's (halves the
        # per-op overhead and semaphore traffic on the phase-1 critical path)
        sel0 = selT[:, 0:1, :].rearrange("p (o a) b -> p o a b", o=1)
        sel13 = selT[:, 1:4, :].rearrange("p (o a) b -> p o a b", o=1)
        qb = None
        for t in range(QD // 2):
            dm = (2 * t) % 8
            if dm == 0:
                qb = qb_batch(t // 4)
            wsls = []
            for dp in range(2):
                wsl = wpool.tile([P, NDC, 2, IS], dt.float8e4, tag="wsl")
                nc.sync.dma_start(wsl[:], wru_d[2 * t + dp])
                wsls.append(wsl)
            qbp = qb[:, dm : dm + 2, :].rearrange("p a (o b) -> p a o b", o=1)
            z4 = z4pool.tile([P, 2, NDC, NOBS], dt.float8e4, tag="z4")
            # chunk 0: DVE fp8-direct; chunks 1-3: DVE bf16 -> ACT fp8
            # (fp8 stores cost ~1.1ns/elem on DVE vs 0.63 bf16 + 0.98 ACT)
            nc.vector.tensor_mul(
                z4[:, :, 0:1, :],
                sel0.to_broadcast([P, 2, 1, NOBS]),
                qbp.to_broadcast([P, 2, 1, NOBS]),
            )
            zb = zbpool.tile([P, 2, 3, NOBS], dt.bfloat16, tag="zb")
            nc.vector.tensor_mul(
                zb[:],
                sel13.to_broadcast([P, 2, 3, NOBS]),
                qbp.to_broadcast([P, 2, 3, NOBS]),
            )
            nc.scalar.copy(z4[:, :, 1:4, :], zb[:])
            # pair-major order: all icp0 MMs (ready right after the DVE
            # fp8-direct op) run while ACT still converts chunks 2-3
            for pair in range(2):
                for dp in range(2):
                    for g in range(2):
                        for oc in range(2):
                            nc.tensor.matmul(
                                acc[(g, oc)][:],
                                wsls[dp][:, 2 * pair : 2 * pair + 2, g, oc * P : (oc + 1) * P],
                                z4[:, dp, 2 * pair : 2 * pair + 2, :],
                                start=False,
                                stop=False,
                                perf_mode=DR,
                            )
        # prefetch phase-2's first qb batch (first full pair is t=PRE) so the
        # phase boundary doesn't stall on a cold 1MB broadcast DMA
        qb_p2 = qb_batch(PRE // 4)
        # tail (i = 512): Ztail = qT * bcast(selTtail); closes the groups
        tb = zbpool.tile([P, NOBS], dt.bfloat16, tag="tb")
        nc.sync.dma_start(tb[:], sttail_d[0:1, :].to_broadcast([P, NOBS]))
        ztail = zbpool.tile([P, NOBS], dt.bfloat16, tag="ztail")
        nc.vector.tensor_mul(ztail[:], qT[:], tb[:])
        for g in range(2):
            for oc in range(2):
                nc.tensor.matmul(
                    acc[(g, oc)][:],
                    wrut[:, g, oc * P : (oc + 1) * P],
                    ztail[:],
                    start=False,
                    stop=True,
                )
        rT = sb.tile([P, 2, NOBS], dt.bfloat16)
        uT = sb.tile([P, 2, NOBS], dt.bfloat16)
        for oc in range(2):
            nc.scalar.activation(
                rT[:, oc, :], acc[(0, oc)][:], AF.Sigmoid, scale=1.0 / (SQ * SW)
            )
            nc.scalar.activation(
                uT[:, oc, :], acc[(1, oc)][:], AF.Sigmoid, scale=1.0 / (SQ * SW)
            )

        # hc = r * h_selT  (hs = CT chunks 2-3, n < 512)
        hc = sb.tile([P, 2, NOBS], dt.bfloat16)
        nc.vector.tensor_mul(hc[:], rT[:], CT[:, 2:4, 0:4, :])
        # precompute w = (1-u)*hc now (DVE is light here) so the output tail
        # is only tanh -> mul -> add
        um = sb.tile([P, 2, NOBS], dt.bfloat16)
        nc.vector.tensor_scalar(um[:], uT[:], -1.0, 1.0, op0=ALU.mult, op1=ALU.add)
        w_uh = sb.tile([P, 2, NOBS], dt.bfloat16)
        nc.vector.tensor_mul(w_uh[:], um[:], hc[:])

        # ---------------- gate phase 2: candidate (bf16) --------------------
        # x-part for the first PRE d-pairs was prepaid during attention
        psF_cm = tc.tile_pool(name="psF", bufs=2, space="PSUM")
        psF = psF_cm.__enter__()
        nc.sync.dma_start(xtail_d[:], ctail[:, 0:4, :])
        hcv = hc[:].rearrange("p (o a) b -> p o a b", o=1)
        qb = None
        # full (x+h) pairs first: their x-part z ops don't depend on hc, so
        # the PE never stalls on the sigmoid->hc chain at the phase boundary.
        # PRE*2 is 8-aligned so qb batch boundaries line up in both regions.
        assert (2 * PRE) % 8 == 0
        for t in list(range(PRE, QD // 2)) + list(range(PRE)):
            dm = (2 * t) % 8
            if dm == 0:
                qb = qb_p2 if t == PRE else qb_batch(t // 4)
            qbp = qb[:, dm : dm + 2, :].rearrange("p a (o b) -> p a o b", o=1)
            qbb = qbp.to_broadcast([P, 2, 2, NOBS])
            z2 = zpool.tile([P, 2, NDC, NOBS], dt.bfloat16, tag="z2")
            if t < PRE:
                wsls = []
                for dp in range(2):
                    wsl = wcpool.tile([P, NDC, IS], dt.bfloat16, tag="wcsl")
                    nc.sync.dma_start(wsl[:, 2:4, :], wc_d[2 * t + dp][:, 2:4, :])
                    wsls.append(wsl)
                nc.vector.tensor_mul(
                    z2[:, :, 2:4, :], hcv.to_broadcast([P, 2, 2, NOBS]), qbb
                )
                ics = (2, 3)
            else:
                wsls = []
                for dp in range(2):
                    wsl = wcpool.tile([P, NDC, IS], dt.bfloat16, tag="wcsl")
                    nc.sync.dma_start(wsl[:], wc_d[2 * t + dp])
                    wsls.append(wsl)
                nc.vector.tensor_mul(
                    z2[:, :, 0:2, :], xsv.to_broadcast([P, 2, 2, NJ // 2, P]), qbb
                )
                nc.vector.tensor_mul(
                    z2[:, :, 2:4, :], hcv.to_broadcast([P, 2, 2, NOBS]), qbb
                )
                ics = (0, 1, 2, 3)
            for dp in range(2):
                for ic in ics:
                    for oc in range(2):
                        nc.tensor.matmul(
                            cacc[oc][:],
                            wsls[dp][:, ic, oc * P : (oc + 1) * P],
                            z2[:, dp, ic, :],
                            start=False,
                            stop=False,
                        )
        ctb = zbpool.tile([P, NOBS], dt.bfloat16, tag="ctb")
        nc.sync.dma_start(ctb[:], xtail_d[0:1, :].to_broadcast([P, NOBS]))
        zctail = zbpool.tile([P, NOBS], dt.bfloat16, tag="zctail")
        nc.vector.tensor_mul(zctail[:], qT[:], ctb[:])
        for oc in range(2):
            nc.tensor.matmul(
                cacc[oc][:],
                wct[:, oc * P : (oc + 1) * P],
                zctail[:],
                start=False,
                stop=True,
            )

        # ---------------- combine + output ----------------
        # out = hc + u * (tanh(cacc) - hc); per-oc so oc0's combine and
        # transposes overlap oc1's tail matmul + tanh
        outf = sb.tile([P, 2, NOBS], dt.float32)
        outT = sb.tile([P, NDC, IS], dt.float32)
        for oc in range(2):
            cand = scratch.tile([P, NOBS], dt.float32, tag="cand")
            nc.scalar.activation(cand[:], cacc[oc][:], AF.Tanh, scale=1.0 / SQ)
            # combine per n-half so the first transposes start while the DVE
            # finishes the second half
            for nh in range(2):
                hsl = slice(nh * IS, (nh + 1) * IS)
                ud = scratch.tile([P, IS], dt.float32, tag="ud")
                nc.vector.tensor_mul(ud[:], cand[:, hsl], uT[:, oc, hsl])
                nc.vector.tensor_add(outf[:, oc, hsl], ud[:], w_uh[:, oc, hsl])
                for ncj in (2 * nh, 2 * nh + 1):
                    pt = psF.tile([P, P], dt.float32, tag="otr")
                    nc.tensor.transpose(
                        pt[:], outf[:, oc, ncj * P : (ncj + 1) * P], id_f32
                    )
                    nc.scalar.copy(outT[:, ncj, oc * P : (oc + 1) * P], pt[:])
                    nc.sync.dma_start(
                        out_d[ncj * P : (ncj + 1) * P, oc * P : (oc + 1) * P],
                        outT[:, ncj, oc * P : (oc + 1) * P],
                    )

        psF_cm.__exit__(None, None, None)
        psD_cm.__exit__(None, None, None)
        psE_cm.__exit__(None, None, None)

    _split_waits(nc, mybir)
    return nc


def _prepare(inputs):
    x = np.asarray(inputs["x"], np.float32)
    h = np.asarray(inputs["h"], np.float32)
    q = np.asarray(inputs["query_vectors"], np.float32)
    attn_w = np.asarray(inputs["attn_w"], np.float32)
    attn_b = np.asarray(inputs["attn_b"], np.float32)
    Wr = np.asarray(inputs["Wr"], np.float32)
    br = np.asarray(inputs["br"], np.float32)
    Wu = np.asarray(inputs["Wu"], np.float32)
    bu = np.asarray(inputs["bu"], np.float32)
    Wc = np.asarray(inputs["Wc"], np.float32)
    bc_ = np.asarray(inputs["bc"], np.float32)
    b_idx = np.asarray(inputs["b_idx"])
    n_idx = np.asarray(inputs["n_idx"])

    assert np.array_equal(b_idx, np.repeat(np.arange(B), NOBS)), "b_idx pattern"
    assert np.array_equal(n_idx, np.tile(np.arange(NOBS), B)), "n_idx pattern"

    cmb = np.concatenate([x, h], axis=-1).astype(BF16)  # [B, N, CD]

    def retile_main(W, dtype):
        # [128, 513, 256] -> rows r of the 512-row main block -> [d, i_lo, ic, o]
        m = W.astype(dtype)
        return m.reshape(QD, NDC, P, IS).transpose(0, 2, 1, 3)

    wr_m = retile_main(Wr[:, :512, :] * SW, F8)
    wu_m = retile_main(Wu[:, :512, :] * SW, F8)
    wru = np.ascontiguousarray(np.stack([wr_m, wu_m], axis=3))  # [d, i_lo, ic, g, o]
    wrut = np.ascontiguousarray(
        (np.stack([Wr[:, 512, :], Wu[:, 512, :]], axis=1) * SW).astype(BF16)
    )
    c_rows = np.r_[0:256, 257:513]
    wc = np.ascontiguousarray(retile_main(Wc[:, c_rows, :], BF16))
    wct = np.ascontiguousarray(Wc[:, 256, :].astype(BF16))
    bru = np.ascontiguousarray((np.stack([br, bu], axis=1) * SW).astype(BF16))
    bcb = np.ascontiguousarray(bc_.astype(BF16))

    # attention weights in C^T-chunk layout: w4t[p, c, k] with k order
    # (tgt0, tgt1, src0, src1); tail = combined col 256
    w_src = attn_w[:, :CD]  # [2, 513]
    w_tgt = attn_w[:, CD:]
    w4 = np.stack([w_tgt[0], w_tgt[1], w_src[0], w_src[1]], axis=1)  # [513, 4]
    w4t = np.zeros((P, NDC, 4), np.float32)
    for c, (lo, hi) in enumerate(CT_COLS):
        w4t[:, c, :] = w4[lo:hi, :]
    w4t = np.ascontiguousarray(w4t.astype(BF16))
    w4tail = np.ascontiguousarray(w4[256:257, :].astype(BF16))
    abv = np.array([[0.0], [0.0], [attn_b[0]], [attn_b[1]]], np.float32)

    in_maps = []
    for b in range(B):
        qTb = np.ascontiguousarray((q[b * NOBS : (b + 1) * NOBS].T * SQ).astype(BF16))
        in_maps.append(
            {
                "cmb": np.ascontiguousarray(cmb[b]),
                "qT": qTb,
                "w4t": w4t,
                "w4tail": w4tail,
                "wru": wru,
                "wrut": wrut,
                "wc": wc,
                "wct": wct,
                "bru": bru,
                "bc": bcb,
                "abv": abv,
            }
        )
    return in_maps, float(attn_b[0]), float(attn_b[1])


def _ensure_ntff_hook():
    """Provide antenv.axon_hooks (missing in this image) so trace=True works."""
    import sys, types, contextlib, ctypes

    try:
        import antenv.axon_hooks  # noqa: F401

        return
    except ImportError:
        pass
    import antenv

    so_path = "/opt/axon/libaxon_pjrt.so"
    hook = None
    try:
        lib = ctypes.CDLL(so_path)
        if hasattr(lib, "axon_start_nrt_profile"):
            lib.axon_start_nrt_profile.argtypes = [
                ctypes.POINTER(ctypes.c_int64),
                ctypes.c_size_t,
            ]
            lib.axon_start_nrt_profile.restype = ctypes.c_int64
            lib.axon_stop_nrt_profile.argtypes = [ctypes.c_char_p]
            lib.axon_stop_nrt_profile.restype = ctypes.c_int64

            @contextlib.contextmanager
            def _hook(output_dir, device_ids):
                import jax

                jax.devices()
                if device_ids:
                    ids = (ctypes.c_int64 * len(device_ids))(*device_ids)
                    rc = lib.axon_start_nrt_profile(ids, len(device_ids))
                else:
                    rc = lib.axon_start_nrt_profile(None, 0)
                if rc != 0:
                    raise RuntimeError(f"axon_start_nrt_profile rc={rc}")
                try:
                    yield
                finally:
                    n = lib.axon_stop_nrt_profile(str(output_dir).encode())
                    print(f"profile: {n} file(s) written to {output_dir}")

            hook = _hook
    except OSError:
        pass

    m = types.ModuleType("antenv.axon_hooks")
    m.get_axon_ntff_profile_hook = lambda: hook
    m.set_axon_ntff_profile_hook = lambda h: None
    sys.modules["antenv.axon_hooks"] = m
    antenv.axon_hooks = m


def _maybe_patch_ldw():
    """Flip walrus --enable-ldw-opt to true (LDWOPT=0 disables)."""
    import os
    import concourse.bass_utils as bu

    if os.environ.get("LDWOPT", "0") != "1" or getattr(bu, "_ldw_patched", False):
        return
    orig = bu.run_command

    def patched(argv, **kwargs):
        argv = [
            a.replace("--enable-ldw-opt=false", "--enable-ldw-opt=true")
            if isinstance(a, str)
            else a
            for a in argv
        ]
        return orig(argv, **kwargs)

    bu.run_command = patched
    bu._ldw_patched = True


def _run(inputs, trace=False):
    import concourse.bass_utils as bu
    from concourse.bass_utils import run_bass_kernel_spmd

    _maybe_patch_ldw()
    if trace:
        _ensure_ntff_hook()
        bu.upload_artifacts = lambda tmpdir: tmpdir

    in_maps, ab0, ab1 = _prepare(inputs)
    key = (ab0, ab1)
    if key not in _CACHE:
        _CACHE[key] = _build(ab0, ab1)
    nc = _CACHE[key]
    bkr = run_bass_kernel_spmd(nc, in_maps, list(range(B)), trace=trace)
    out = np.concatenate([np.asarray(bkr.results[b]["out"]) for b in range(B)], axis=0)
    return out.astype(np.float32), bkr


def kernel(**inputs) -> np.ndarray:
    return _run(inputs, trace=False)[0]


# revision 59
# speedup vs baseline: 1.0082x; 1.0082x over previous
"""Trainium2 Bass kernel for AGATCellWithMLP (gnn_message_passing).

Data-parallel across 8 NeuronCores: core b owns graph b (B=8).  Hypernetwork
weights are replicated and streamed; no collectives.

Math (per core, NOBS=512 selected rows n, CD=513 features, IS=256 out):
  combined = [x | h]                                  [1024, 513]
  scoresT[j, n] = leaky(src[n] + tgt[j] + ab)          per head
  attnT[j, n]  = exp(scoresT) * (0.5 / colsum)        (softmax + head-mean)
  selT[d, n]   = sum_h (combined^T @ attnT_h)         [513, 512]
  g(W, inT)[o, n] = sum_{d,i} qT[d,n] * inT[i,n] * W[d,i,o] + (b^T q)[o,n]
  r = sigmoid(g(Wr, selT)); u = sigmoid(g(Wu, selT))
  cT = [x_selT | r*h_selT]
  cand = tanh(g(Wc, cT))
  out[o, n] = (1-u)*r*h_selT + u*cand   -> transpose -> [512, 256]

Performance structure (from trace analysis):
 - The PE runs matmuls back-to-back at ~217ns/MM ([128c,128o,512n], LDWEIGHTS
   hidden) when its queue has no unsatisfied waits; the kernel is built so
   every stage keeps producers >=1 iteration ahead of the PE.
 - Attention is computed almost entirely on the PE: C^T via transposes
   (reused later as xs/hs), src/tgt as a [128d x 4] matmul over C^T, and
   scoresT[j,n] = src[n] + tgt[j] as a rank-3 matmul ([tgt_hi; tgt_lo; ones]
   x [ones; ones; src]) so the DVE only does the leaky-relu.
 - r/u contraction runs in fp8 e4m3 DoubleRow.  The per-d z = selT * q_d
   fp8 production (the old bottleneck: fp8 stores cost ~2.4ns/elem on DVE)
   is split: DVE writes chunks 0-1 fp8-direct (merged op ~1.4ns/elem), and
   produces chunks 2-3 in bf16 (~0.73ns/elem) which the otherwise-idle
   scalar engine converts to fp8.
 - q_d broadcast rows are DMA'd in batches of 8 d's (one descriptor set per
   batch instead of per d).

Precision: host pre-scales qT by SQ=8 and the r/u weight set by SW=64 to
center fp8 e4m3 dynamic range; the device unscales inside the sigmoid.  The
candidate gate stays bf16 (tanh preact std ~13; fp8 fails the 2e-2 gate).
"""

import numpy as np
import ml_dtypes

BF16 = ml_dtypes.bfloat16
F8 = ml_dtypes.float8_e4m3

B, N, IS, QD, H = 8, 1024, 256, 128, 2
CD = 2 * IS + 1  # 513
NOBS = N // 2  # 512
K = B * NOBS  # 4096
P = 128
NJ = N // P  # 8 j-chunks
NDC = 4  # full d-chunks of the 513 contraction (tail row handled separately)

SQ = 8.0  # host scale on qT (folded out in the gate activations)
SW = 64.0  # host scale on the r/u weight set (fp8 dynamic-range centering)

# shifted column chunks of `combined` used for C^T / xs / hs (tail col 256
# handled separately so x rows [0:256) and h rows [257:513) stay aligned)
CT_COLS = [(0, 128), (128, 256), (257, 385), (385, 513)]

_CACHE = {}


def _split_waits(nc, mybir):
    # This toolchain's walrus allows only ONE sync-wait per instruction
    # ("Too many sync wait commands"); hoist extra waits onto standalone
    # same-engine EventSemaphore nops placed immediately before.
    n = 0
    for f in nc.m.functions:
        for blk in f.blocks:
            out = []
            for inst in blk.instructions:
                si = inst.sync_info
                if si is not None and si.on_wait and len(si.on_wait) > 1:
                    waits = list(si.on_wait)
                    for w in waits[:-1]:
                        ev = mybir.InstEventSemaphore(
                            name=f"I-wsplit-{n}", engine=inst.engine, ins=[], outs=[]
                        )
                        ev.sync_info = mybir.SyncInfo(on_wait=[w], on_update=[])
                        out.append(ev)
                        n += 1
                    inst.sync_info = mybir.SyncInfo(
                        on_wait=[waits[-1]], on_update=list(si.on_update or [])
                    )
                out.append(inst)
            blk.instructions = out


def _build(ab0: float, ab1: float):
    import concourse.bass as bass
    import concourse.mybir as mybir
    import concourse.tile as tile
    from concourse.masks import make_identity
    from contextlib import ExitStack

    dt = mybir.dt
    AF = mybir.ActivationFunctionType
    ALU = mybir.AluOpType
    DR = mybir.MatmulPerfMode.DoubleRow

    nc = bass.Bass()
    # The kernel-tail EVENT_SEMAPHORE_RANGE_CLEAR (InstISA) is rejected by this
    # toolchain's walrus ("ISA wrong length"); the NEFF runs once per load, so
    # skipping the tail semaphore clear is safe.
    nc.clear_and_free_semaphores = lambda sems: None

    cmb_d = nc.declare_dram_parameter("cmb", [N, CD], dt.bfloat16, isOutput=False)
    qT_d = nc.declare_dram_parameter("qT", [QD, NOBS], dt.bfloat16, isOutput=False)
    w4t_d = nc.declare_dram_parameter("w4t", [P, NDC, 4], dt.bfloat16, isOutput=False)
    w4tail_d = nc.declare_dram_parameter("w4tail", [1, 4], dt.bfloat16, isOutput=False)
    wru_d = nc.declare_dram_parameter(
        "wru", [QD, P, NDC, 2, IS], dt.float8e4, isOutput=False
    )
    wrut_d = nc.declare_dram_parameter("wrut", [QD, 2, IS], dt.bfloat16, isOutput=False)
    wc_d = nc.declare_dram_parameter("wc", [QD, P, NDC, IS], dt.bfloat16, isOutput=False)
    wct_d = nc.declare_dram_parameter("wct", [QD, IS], dt.bfloat16, isOutput=False)
    bru_d = nc.declare_dram_parameter("bru", [QD, 2, IS], dt.bfloat16, isOutput=False)
    bc_d = nc.declare_dram_parameter("bc", [QD, IS], dt.bfloat16, isOutput=False)
    abv_d = nc.declare_dram_parameter("abv", [4, 1], dt.float32, isOutput=False)
    out_d = nc.declare_dram_parameter("out", [NOBS, IS], dt.float32, isOutput=True)
    # DRAM bounce buffers (partition remaps / broadcasts)
    sttail_d = nc.dram_tensor("sttail_dram", [1, NOBS], dt.bfloat16)
    xtail_d = nc.dram_tensor("xtail_dram", [1, NOBS], dt.bfloat16)
    tgthi_d = nc.dram_tensor("tgthi_dram", [H, N], dt.bfloat16)
    tgtlo_d = nc.dram_tensor("tgtlo_dram", [H, N], dt.bfloat16)
    src_d = nc.dram_tensor("src_dram", [H, NOBS], dt.bfloat16)

    with tile.TileContext(nc) as tc, ExitStack() as ctx:
        consts = ctx.enter_context(tc.tile_pool(name="consts", bufs=1))
        sb = ctx.enter_context(tc.tile_pool(name="sb", bufs=1))
        scratch = ctx.enter_context(tc.tile_pool(name="scratch", bufs=2))
        qbpool = ctx.enter_context(tc.tile_pool(name="qbpool", bufs=3))
        e2pool = ctx.enter_context(tc.tile_pool(name="e2pool", bufs=3))
        zpool = ctx.enter_context(tc.tile_pool(name="zpool", bufs=2))
        z4pool = ctx.enter_context(tc.tile_pool(name="z4pool", bufs=3))
        zbpool = ctx.enter_context(tc.tile_pool(name="zbpool", bufs=2))
        wpool = ctx.enter_context(tc.tile_pool(name="wpool", bufs=6))
        wcpool = ctx.enter_context(tc.tile_pool(name="wcpool", bufs=6))
        wcxpool = ctx.enter_context(tc.tile_pool(name="wcxpool", bufs=4))

        # ---------------- constants ----------------
        id_bf = consts.tile([P, P], dt.bfloat16)
        make_identity(nc, id_bf)
        id_f32 = consts.tile([P, P], dt.float32)
        make_identity(nc, id_f32)
        ones_col = consts.tile([P, 1], dt.bfloat16)
        nc.gpsimd.memset(ones_col, 1.0)
        ones_row = consts.tile([1, P], dt.bfloat16)
        nc.gpsimd.memset(ones_row, 1.0)

        # ---------------- inputs to SBUF ----------------
        C = sb.tile([P, NJ, CD], dt.bfloat16)  # combined, j-chunked
        # per-jc DMAs so the first transposes start as soon as chunk 0 lands
        for jc in range(NJ):
            nc.sync.dma_start(C[:, jc, :], cmb_d[jc * P : (jc + 1) * P, :])
        qT = sb.tile([P, NOBS], dt.bfloat16)
        nc.sync.dma_start(qT[:], qT_d[:])
        w4t = sb.tile([P, NDC, 4], dt.bfloat16)
        nc.sync.dma_start(w4t[:], w4t_d[:])
        w4tail = sb.tile([1, 4], dt.bfloat16)
        nc.sync.dma_start(w4tail[:], w4tail_d[:])
        wrut = sb.tile([P, 2, IS], dt.bfloat16)
        nc.sync.dma_start(wrut[:], wrut_d[:])
        wct = sb.tile([P, IS], dt.bfloat16)
        nc.sync.dma_start(wct[:], wct_d[:])
        bru = sb.tile([P, 2, IS], dt.bfloat16)
        nc.sync.dma_start(bru[:], bru_d[:])
        bc = sb.tile([P, IS], dt.bfloat16)
        nc.sync.dma_start(bc[:], bc_d[:])
        abv = sb.tile([4, 1], dt.float32)
        nc.sync.dma_start(abv[:], abv_d[:])

        # ---------------- c-gate accumulators (opened early: the x-part
        # matmuls are independent of attention and prepaid into its PE idle)
        psE_cm = tc.tile_pool(name="psE", bufs=1, space="PSUM")
        psE = psE_cm.__enter__()
        cacc = [
            psE.tile([P, NOBS], dt.float32, tag=f"cacc{oc}", name=f"cacc{oc}")
            for oc in range(2)
        ]
        for oc in range(2):
            nc.tensor.matmul(
                cacc[oc][:], bc[:, oc * P : (oc + 1) * P], qT[:], start=True, stop=False
            )

        def qb_batch(t8):
            qbt = qbpool.tile([P, 8, NOBS], dt.bfloat16, tag="qb8")
            nc.sync.dma_start(
                qbt[:],
                qT_d[t8 * 8 : t8 * 8 + 8, :]
                .rearrange("(o a) b -> o a b", o=1)
                .to_broadcast([P, 8, NOBS]),
            )
            return qbt

        # ---------------- C^T (all j), tails ----------------
        CT = sb.tile([P, NDC, NJ, P], dt.bfloat16)  # shifted chunks, see CT_COLS
        ctail = sb.tile([1, NJ, P], dt.bfloat16)  # combined col 256 per jc
        xsv = CT[:, 0:2, 0:4, :].rearrange("p (o a) jc b -> p o a jc b", o=1)

        PRE = 20  # c-gate x-part d-pairs prepaid during attention
        _pre = {"t": 0, "qb": None}

        def prepay_pair():
            t = _pre["t"]
            if t >= PRE:
                return
            _pre["t"] += 1
            dm = (2 * t) % 8
            if dm == 0:
                _pre["qb"] = qb_batch(t // 4)
            qbx = _pre["qb"]
            wcxs = []
            for dp in range(2):
                w = wcxpool.tile([P, 2, IS], dt.bfloat16, tag="wcx")
                nc.sync.dma_start(w[:], wc_d[2 * t + dp][:, 0:2, :])
                wcxs.append(w)
            qbp = qbx[:, dm : dm + 2, :].rearrange("p a (o b) -> p a o b", o=1)
            xq = zpool.tile([P, 2, NDC, NOBS], dt.bfloat16, tag="z2")
            nc.vector.tensor_mul(
                xq[:, :, 0:2, :],
                xsv.to_broadcast([P, 2, 2, NJ // 2, P]),
                qbp.to_broadcast([P, 2, 2, NOBS]),
            )
            for dp in range(2):
                for ic in range(2):
                    for oc in range(2):
                        nc.tensor.matmul(
                            cacc[oc][:],
                            wcxs[dp][:, ic, oc * P : (oc + 1) * P],
                            xq[:, dp, ic, :],
                            start=False,
                            stop=False,
                        )

        psT_cm = tc.tile_pool(name="psT", bufs=2, space="PSUM")
        psT = psT_cm.__enter__()

        def transpose_jc(jc):
            tp = psT.tile([P, NDC, P], dt.bfloat16, tag="tp")
            for c, (lo, hi) in enumerate(CT_COLS):
                nc.tensor.transpose(tp[:, c, :], C[:, jc, lo:hi], id_bf)
            # alternate ACT/DVE so the copy chain isn't serial on one engine
            if jc % 2 == 0:
                nc.scalar.copy(CT[:, :, jc, :], tp[:])
            else:
                nc.vector.tensor_copy(CT[:, :, jc, :], tp[:])
            tq = psT.tile([1, P], dt.bfloat16, tag="tq")
            nc.tensor.transpose(tq[:], C[:, jc, 256:257], id_bf)
            nc.scalar.copy(ctail[:, jc, :], tq[:])

        for jc in range(4):
            transpose_jc(jc)
        prepay_pair()  # xs (CT jc 0-3, chunks 0-1) is ready now
        prepay_pair()
        for jc in range(4, NJ):
            transpose_jc(jc)

        # ---------------- src/tgt via PE ----------------
        # A[k, j] for k in (tgt0, tgt1, src0, src1); two j-halves of 512
        As = sb.tile([4, 2, NOBS], dt.bfloat16)
        Alo = sb.tile([2, 2, NOBS], dt.bfloat16)
        psA_cm = tc.tile_pool(name="psA", bufs=2, space="PSUM")
        psA = psA_cm.__enter__()
        for half in range(2):
            A = psA.tile([4, NOBS], dt.float32, tag="A")
            for c in range(NDC):
                nc.tensor.matmul(
                    A[:],
                    w4t[:, c, :],
                    CT[:, c, half * 4 : half * 4 + 4, :],
                    start=(c == 0),
                    stop=False,
                )
            nc.tensor.matmul(
                A[:],
                w4tail[:],
                ctail[:, half * 4 : half * 4 + 4, :],
                start=False,
                stop=True,
            )
            # cast (+attn_b on the src rows); keep a bf16-lo residual for tgt
            nc.scalar.activation(As[:, half, :], A[:], AF.Identity, bias=abv[:])
            nc.vector.tensor_sub(Alo[:, half, :], A[0:2, :], As[0:2, half, :])

        # partition remap via DRAM bounce: TT = [tgt_hi; tgt_lo; ones],
        # RR = [ones; ones; src+ab]
        TT = sb.tile([3, H, NJ, P], dt.bfloat16)
        RR = sb.tile([3, H, NOBS], dt.bfloat16)
        # memset whole tiles (partition base must be 0); DMAs below overwrite
        # the non-ones rows
        nc.gpsimd.memset(TT[:], 1.0)
        nc.gpsimd.memset(RR[:], 1.0)
        for hh in range(H):
            for half in range(2):
                nc.sync.dma_start(
                    tgthi_d[hh : hh + 1, half * NOBS : (half + 1) * NOBS],
                    As[hh : hh + 1, half, :],
                )
                nc.sync.dma_start(
                    tgtlo_d[hh : hh + 1, half * NOBS : (half + 1) * NOBS],
                    Alo[hh : hh + 1, half, :],
                )
            nc.sync.dma_start(src_d[hh : hh + 1, :], As[2 + hh : 3 + hh, 0, :])
        for hh in range(H):
            nc.sync.dma_start(
                TT[0:1, hh, :, :], tgthi_d[hh : hh + 1, :].rearrange("o (jc p) -> o jc p", p=P)
            )
            nc.sync.dma_start(
                TT[1:2, hh, :, :], tgtlo_d[hh : hh + 1, :].rearrange("o (jc p) -> o jc p", p=P)
            )
            nc.sync.dma_start(RR[2:3, hh, :], src_d[hh : hh + 1, :])
        psA_cm.__exit__(None, None, None)
        psT_cm.__exit__(None, None, None)
        prepay_pair()
        prepay_pair()

        # ---------------- scores -> exp (+ column sums) ----------------
        expT = sb.tile([P, H, NJ, NOBS], dt.bfloat16)
        psZ_cm = tc.tile_pool(name="psZ", bufs=1, space="PSUM")
        psZ = psZ_cm.__enter__()
        psS_cm = tc.tile_pool(name="psS", bufs=4, space="PSUM")
        psS = psS_cm.__enter__()
        zt = []
        for hh in range(H):
            zth = psZ.tile([1, NOBS], dt.float32, tag=f"zt{hh}", name=f"zt{hh}")
            zt.append(zth)
            for jc in range(NJ):
                sc = psS.tile([P, NOBS], dt.float32, tag="sc")
                nc.tensor.matmul(
                    sc[:], TT[:, hh, jc, :], RR[:, hh, :], start=True, stop=True
                )
                # exp(leaky(x)) == max(exp(x), exp(0.2x)) — avoids a PSUM-read
                # TensorScalarPtr (BIR verifier rejects it) and ACT table swaps
                nc.scalar.activation(expT[:, hh, jc, :], sc[:], AF.Exp)
                e2 = e2pool.tile([P, NOBS], dt.bfloat16, tag="e2")
                nc.scalar.activation(e2[:], sc[:], AF.Exp, scale=0.2)
                nc.vector.tensor_max(expT[:, hh, jc, :], expT[:, hh, jc, :], e2[:])
                nc.tensor.matmul(
                    zth[:],
                    ones_col[:],
                    expT[:, hh, jc, :],
                    start=(jc == 0),
                    stop=(jc == NJ - 1),
                )
                prepay_pair()
        psS_cm.__exit__(None, None, None)

        # ---------------- invZ = 0.5/colsum, broadcast to [128, n] ----------
        # reciprocal on a [128, 4] transposed layout (DVE recip is per-lane
        # serial: [1,512] costs ~4us, [128,4] is ~free)
        psI_cm = tc.tile_pool(name="psI", bufs=1, space="PSUM")
        psI = psI_cm.__enter__()
        invZB = sb.tile([P, H, NOBS], dt.bfloat16)
        for hh in range(H):
            ztsb = scratch.tile([1, NOBS], dt.float32, tag="ztsb")
            nc.scalar.activation(ztsb[:], zt[hh][:], AF.Copy, scale=2.0)
            ztc = psI.tile([P, NDC], dt.float32, tag="ztc")
            for c in range(NDC):
                nc.tensor.transpose(
                    ztc[:, c : c + 1], ztsb[:, c * P : (c + 1) * P], id_f32[0:1, 0:1]
                )
            ztcs = scratch.tile([P, NDC], dt.float32, tag="ztcs")
            nc.scalar.copy(ztcs[:], ztc[:])
            izc = scratch.tile([P, NDC], dt.float32, tag="izc")
            nc.vector.reciprocal(izc[:], ztcs[:])
            izr = psI.tile([1, NOBS], dt.float32, tag="izr")
            for c in range(NDC):
                nc.tensor.transpose(izr[:, c * P : (c + 1) * P], izc[:, c : c + 1], id_f32)
            izrb = scratch.tile([1, NOBS], dt.bfloat16, tag="izrb")
            nc.scalar.copy(izrb[:], izr[:])
            ib = psI.tile([P, NOBS], dt.float32, tag="ib")
            nc.tensor.matmul(ib[:], ones_row[:], izrb[:], start=True, stop=True)
            nc.scalar.copy(invZB[:, hh, :], ib[:])
        psI_cm.__exit__(None, None, None)
        psZ_cm.__exit__(None, None, None)

        # ---------------- attnT = expT * invZB ----------------
        attnT = sb.tile([P, H, NJ, NOBS], dt.bfloat16)
        for hh in range(H):
            nc.vector.tensor_mul(
                attnT[:, hh, :, :],
                expT[:, hh, :, :],
                invZB[:, hh : hh + 1, :].to_broadcast([P, NJ, NOBS]),
            )
            prepay_pair()

        # ---------------- selT = C^T @ attnT (summed over heads) -----------
        selT = sb.tile([P, NDC, NOBS], dt.bfloat16)
        selTtail = sb.tile([1, NOBS], dt.bfloat16)
        psL_cm = tc.tile_pool(name="psL", bufs=1, space="PSUM")
        psL = psL_cm.__enter__()
        selps = []
        for dc in range(NDC):
            sp = psL.tile([P, NOBS], dt.float32, tag=f"sel{dc}", name=f"sel{dc}")
            selps.append(sp)
            first = True
            for hh in range(H):
                for jc in range(NJ):
                    nc.tensor.matmul(
                        sp[:],
                        C[:, jc, dc * P : (dc + 1) * P],
                        attnT[:, hh, jc, :],
                        start=first,
                        stop=(hh == H - 1 and jc == NJ - 1),
                    )
                    first = False
            if dc % 2 == 0:
                nc.scalar.copy(selT[:, dc, :], sp[:])
            else:
                nc.vector.tensor_copy(selT[:, dc, :], sp[:])
            prepay_pair()
        zq = psL.tile([1, NOBS], dt.float32, tag="zq", name="zq")
        first = True
        for hh in range(H):
            for jc in range(NJ):
                nc.tensor.matmul(
                    zq[:],
                    C[:, jc, 512:513],
                    attnT[:, hh, jc, :],
                    start=first,
                    stop=(hh == H - 1 and jc == NJ - 1),
                )
                first = False
        nc.scalar.copy(selTtail[:], zq[:])
        qb1_0 = qb_batch(0)  # phase-1's first broadcast batch, warmed early
        for _ in range(6):
            prepay_pair()
        psL_cm.__exit__(None, None, None)

        # ---------------- gate phase 1: r and u (fp8 DoubleRow) -------------
        psD_cm = tc.tile_pool(name="psD", bufs=1, space="PSUM")
        psD = psD_cm.__enter__()
        acc = {}
        for g in range(2):
            for oc in range(2):
                acc[(g, oc)] = psD.tile(
                    [P, NOBS], dt.float32, tag=f"acc{g}{oc}", name=f"acc{g}{oc}"
                )
        for g in range(2):
            for oc in range(2):
                nc.tensor.matmul(
                    acc[(g, oc)][:],
                    bru[:, g, oc * P : (oc + 1) * P],
                    qT[:],
                    start=True,
                    stop=False,
                )
        nc.sync.dma_start(sttail_d[:], selTtail[:])
        # tail operand prepared up front so the group-closing matmuls after
        # the d-loop never wait on the DRAM bounce roundtrip
        tb = zbpool.tile([P, NOBS], dt.bfloat16, tag="tb")
        nc.sync.dma_start(tb[:], sttail_d[0:1, :].to_broadcast([P, NOBS]))
        ztail = zbpool.tile([P, NOBS], dt.bfloat16, tag="ztail")
        nc.vector.tensor_mul(ztail[:], qT[:], tb[:])

        # d-pair batched z production: one DVE op covers two d's (halves the
        # per-op overhead and semaphore traffic on the phase-1 critical path)
        sel0 = selT[:, 0:1, :].rearrange("p (o a) b -> p o a b", o=1)
        sel13 = selT[:, 1:4, :].rearrange("p (o a) b -> p o a b", o=1)
        qb = None
        for t in range(QD // 2):
            dm = (2 * t) % 8
            if dm == 0:
                qb = qb1_0 if t == 0 else qb_batch(t // 4)
            wsls = []
            for dp in range(2):
                wsl = wpool.tile([P, NDC, 2, IS], dt.float8e4, tag="wsl")
                nc.sync.dma_start(wsl[:], wru_d[2 * t + dp])
                wsls.append(wsl)
            qbp = qb[:, dm : dm + 2, :].rearrange("p a (o b) -> p a o b", o=1)
            z4 = z4pool.tile([P, 2, NDC, NOBS], dt.float8e4, tag="z4")
            # chunk 0: DVE fp8-direct; chunks 1-3: DVE bf16 -> ACT fp8
            # (fp8 stores cost ~1.1ns/elem on DVE vs 0.63 bf16 + 0.98 ACT)
            nc.vector.tensor_mul(
                z4[:, :, 0:1, :],
                sel0.to_broadcast([P, 2, 1, NOBS]),
                qbp.to_broadcast([P, 2, 1, NOBS]),
            )
            zb = zbpool.tile([P, 2, 3, NOBS], dt.bfloat16, tag="zb")
            nc.vector.tensor_mul(
                zb[:],
                sel13.to_broadcast([P, 2, 3, NOBS]),
                qbp.to_broadcast([P, 2, 3, NOBS]),
            )
            nc.scalar.copy(z4[:, :, 1:4, :], zb[:])
            # pair-major order: all icp0 MMs (ready right after the DVE
            # fp8-direct op) run while ACT still converts chunks 2-3
            for pair in range(2):
                for dp in range(2):
                    for g in range(2):
                        for oc in range(2):
                            nc.tensor.matmul(
                                acc[(g, oc)][:],
                                wsls[dp][:, 2 * pair : 2 * pair + 2, g, oc * P : (oc + 1) * P],
                                z4[:, dp, 2 * pair : 2 * pair + 2, :],
                                start=False,
                                stop=False,
                                perf_mode=DR,
                            )
        # prefetch phase-2's first qb batch (first full pair is t=PRE) so the
        # phase boundary doesn't stall on a cold 1MB broadcast DMA
        qb_p2 = qb_batch(PRE // 4)
        # tail (i = 512): Ztail = qT * bcast(selTtail); closes the groups
        for g in range(2):
            for oc in range(2):
                nc.tensor.matmul(
                    acc[(g, oc)][:],
                    wrut[:, g, oc * P : (oc + 1) * P],
                    ztail[:],
                    start=False,
                    stop=True,
                )
        rT = sb.tile([P, 2, NOBS], dt.bfloat16)
        uT = sb.tile([P, 2, NOBS], dt.bfloat16)
        for oc in range(2):
            nc.scalar.activation(
                rT[:, oc, :], acc[(0, oc)][:], AF.Sigmoid, scale=1.0 / (SQ * SW)
            )
            nc.scalar.activation(
                uT[:, oc, :], acc[(1, oc)][:], AF.Sigmoid, scale=1.0 / (SQ * SW)
            )

        # preload the Tanh ACT table now (otherwise the output drain pays the
        # ~1.5us table swap after the sigmoids)
        warm = scratch.tile([1, 8], dt.float32, tag="warm")
        nc.scalar.activation(warm[:], qT[0:1, 0:8], AF.Tanh)
        # hc = r * h_selT  (hs = CT chunks 2-3, n < 512)
        hc = sb.tile([P, 2, NOBS], dt.bfloat16)
        nc.vector.tensor_mul(hc[:], rT[:], CT[:, 2:4, 0:4, :])
        # precompute w = (1-u)*hc now (DVE is light here) so the output tail
        # is only tanh -> mul -> add
        um = sb.tile([P, 2, NOBS], dt.bfloat16)
        nc.vector.tensor_scalar(um[:], uT[:], -1.0, 1.0, op0=ALU.mult, op1=ALU.add)
        w_uh = sb.tile([P, 2, NOBS], dt.bfloat16)
        nc.vector.tensor_mul(w_uh[:], um[:], hc[:])

        # ---------------- gate phase 2: candidate (bf16) --------------------
        # x-part for the first PRE d-pairs was prepaid during attention
        psF_cm = tc.tile_pool(name="psF", bufs=2, space="PSUM")
        psF = psF_cm.__enter__()
        nc.sync.dma_start(xtail_d[:], ctail[:, 0:4, :])
        ctb = zbpool.tile([P, NOBS], dt.bfloat16, tag="ctb")
        nc.sync.dma_start(ctb[:], xtail_d[0:1, :].to_broadcast([P, NOBS]))
        zctail = zbpool.tile([P, NOBS], dt.bfloat16, tag="zctail")
        nc.vector.tensor_mul(zctail[:], qT[:], ctb[:])
        hcv = hc[:].rearrange("p (o a) b -> p o a b", o=1)
        qb = None
        # full (x+h) pairs first: their x-part z ops don't depend on hc, so
        # the PE never stalls on the sigmoid->hc chain at the phase boundary.
        # PRE*2 is 8-aligned so qb batch boundaries line up in both regions.
        assert (2 * PRE) % 8 == 0
        for t in list(range(PRE, QD // 2)) + list(range(PRE)):
            dm = (2 * t) % 8
            if dm == 0:
                qb = qb_p2 if t == PRE else qb_batch(t // 4)
            qbp = qb[:, dm : dm + 2, :].rearrange("p a (o b) -> p a o b", o=1)
            qbb = qbp.to_broadcast([P, 2, 2, NOBS])
            z2 = zpool.tile([P, 2, NDC, NOBS], dt.bfloat16, tag="z2")
            if t < PRE:
                wsls = []
                for dp in range(2):
                    wsl = wcpool.tile([P, NDC, IS], dt.bfloat16, tag="wcsl")
                    nc.sync.dma_start(wsl[:, 2:4, :], wc_d[2 * t + dp][:, 2:4, :])
                    wsls.append(wsl)
                nc.vector.tensor_mul(
                    z2[:, :, 2:4, :], hcv.to_broadcast([P, 2, 2, NOBS]), qbb
                )
                ics = (2, 3)
            else:
                wsls = []
                for dp in range(2):
                    wsl = wcpool.tile([P, NDC, IS], dt.bfloat16, tag="wcsl")
                    nc.sync.dma_start(wsl[:], wc_d[2 * t + dp])
                    wsls.append(wsl)
                nc.vector.tensor_mul(
                    z2[:, :, 0:2, :], xsv.to_broadcast([P, 2, 2, NJ // 2, P]), qbb
                )
                nc.vector.tensor_mul(
                    z2[:, :, 2:4, :], hcv.to_broadcast([P, 2, 2, NOBS]), qbb
                )
                ics = (0, 1, 2, 3)
            for dp in range(2):
                for ic in ics:
                    for oc in range(2):
                        nc.tensor.matmul(
                            cacc[oc][:],
                            wsls[dp][:, ic, oc * P : (oc + 1) * P],
                            z2[:, dp, ic, :],
                            start=False,
                            stop=False,
                        )
        for oc in range(2):
            nc.tensor.matmul(
                cacc[oc][:],
                wct[:, oc * P : (oc + 1) * P],
                zctail[:],
                start=False,
                stop=True,
            )

        # ---------------- combine + output ----------------
        # out = hc + u * (tanh(cacc) - hc); per-oc so oc0's combine and
        # transposes overlap oc1's tail matmul + tanh
        outf = sb.tile([P, 2, NOBS], dt.float32)
        outT = sb.tile([P, NDC, IS], dt.float32)
        for oc in range(2):
            cand = scratch.tile([P, NOBS], dt.float32, tag="cand")
            nc.scalar.activation(cand[:], cacc[oc][:], AF.Tanh, scale=1.0 / SQ)
            # combine per n-half so the first transposes start while the DVE
            # finishes the second half
            for nh in range(2):
                hsl = slice(nh * IS, (nh + 1) * IS)
                ud = scratch.tile([P, IS], dt.float32, tag="ud")
                nc.vector.tensor_mul(ud[:], cand[:, hsl], uT[:, oc, hsl])
                nc.vector.tensor_add(outf[:, oc, hsl], ud[:], w_uh[:, oc, hsl])
                for ncj in (2 * nh, 2 * nh + 1):
                    pt = psF.tile([P, P], dt.float32, tag="otr")
                    nc.tensor.transpose(
                        pt[:], outf[:, oc, ncj * P : (ncj + 1) * P], id_f32
                    )
                    nc.scalar.copy(outT[:, ncj, oc * P : (oc + 1) * P], pt[:])
                    nc.sync.dma_start(
                        out_d[ncj * P : (ncj + 1) * P, oc * P : (oc + 1) * P],
                        outT[:, ncj, oc * P : (oc + 1) * P],
                    )

        psF_cm.__exit__(None, None, None)
        psD_cm.__exit__(None, None, None)
        psE_cm.__exit__(None, None, None)

    _split_waits(nc, mybir)
    return nc


def _prepare(inputs):
    x = np.asarray(inputs["x"], np.float32)
    h = np.asarray(inputs["h"], np.float32)
    q = np.asarray(inputs["query_vectors"], np.float32)
    attn_w = np.asarray(inputs["attn_w"], np.float32)
    attn_b = np.asarray(inputs["attn_b"], np.float32)
    Wr = np.asarray(inputs["Wr"], np.float32)
    br = np.asarray(inputs["br"], np.float32)
    Wu = np.asarray(inputs["Wu"], np.float32)
    bu = np.asarray(inputs["bu"], np.float32)
    Wc = np.asarray(inputs["Wc"], np.float32)
    bc_ = np.asarray(inputs["bc"], np.float32)
    b_idx = np.asarray(inputs["b_idx"])
    n_idx = np.asarray(inputs["n_idx"])

    assert np.array_equal(b_idx, np.repeat(np.arange(B), NOBS)), "b_idx pattern"
    assert np.array_equal(n_idx, np.tile(np.arange(NOBS), B)), "n_idx pattern"

    cmb = np.concatenate([x, h], axis=-1).astype(BF16)  # [B, N, CD]

    def retile_main(W, dtype):
        # [128, 513, 256] -> rows r of the 512-row main block -> [d, i_lo, ic, o]
        m = W.astype(dtype)
        return m.reshape(QD, NDC, P, IS).transpose(0, 2, 1, 3)

    wr_m = retile_main(Wr[:, :512, :] * SW, F8)
    wu_m = retile_main(Wu[:, :512, :] * SW, F8)
    wru = np.ascontiguousarray(np.stack([wr_m, wu_m], axis=3))  # [d, i_lo, ic, g, o]
    wrut = np.ascontiguousarray(
        (np.stack([Wr[:, 512, :], Wu[:, 512, :]], axis=1) * SW).astype(BF16)
    )
    c_rows = np.r_[0:256, 257:513]
    wc = np.ascontiguousarray(retile_main(Wc[:, c_rows, :], BF16))
    wct = np.ascontiguousarray(Wc[:, 256, :].astype(BF16))
    bru = np.ascontiguousarray((np.stack([br, bu], axis=1) * SW).astype(BF16))
    bcb = np.ascontiguousarray(bc_.astype(BF16))

    # attention weights in C^T-chunk layout: w4t[p, c, k] with k order
    # (tgt0, tgt1, src0, src1); tail = combined col 256
    w_src = attn_w[:, :CD]  # [2, 513]
    w_tgt = attn_w[:, CD:]
    w4 = np.stack([w_tgt[0], w_tgt[1], w_src[0], w_src[1]], axis=1)  # [513, 4]
    w4t = np.zeros((P, NDC, 4), np.float32)
    for c, (lo, hi) in enumerate(CT_COLS):
        w4t[:, c, :] = w4[lo:hi, :]
    w4t = np.ascontiguousarray(w4t.astype(BF16))
    w4tail = np.ascontiguousarray(w4[256:257, :].astype(BF16))
    abv = np.array([[0.0], [0.0], [attn_b[0]], [attn_b[1]]], np.float32)

    in_maps = []
    for b in range(B):
        qTb = np.ascontiguousarray((q[b * NOBS : (b + 1) * NOBS].T * SQ).astype(BF16))
        in_maps.append(
            {
                "cmb": np.ascontiguousarray(cmb[b]),
                "qT": qTb,
                "w4t": w4t,
                "w4tail": w4tail,
                "wru": wru,
                "wrut": wrut,
                "wc": wc,
                "wct": wct,
                "bru": bru,
                "bc": bcb,
                "abv": abv,
            }
        )
    return in_maps, float(attn_b[0]), float(attn_b[1])


def _ensure_ntff_hook():
    """Provide antenv.axon_hooks (missing in this image) so trace=True works."""
    import sys, types, contextlib, ctypes

    try:
        import antenv.axon_hooks  # noqa: F401

        return
    except ImportError:
        pass
    import antenv

    so_path = "/opt/axon/libaxon_pjrt.so"
    hook = None
    try:
        lib = ctypes.CDLL(so_path)
        if hasattr(lib, "axon_start_nrt_profile"):
            lib.axon_start_nrt_profile.argtypes = [
                ctypes.POINTER(ctypes.c_int64),
                ctypes.c_size_t,
            ]
            lib.axon_start_nrt_profile.restype = ctypes.c_int64
            lib.axon_stop_nrt_profile.argtypes = [ctypes.c_char_p]
            lib.axon_stop_nrt_profile.restype = ctypes.c_int64

            @contextlib.contextmanager
            def _hook(output_dir, device_ids):
                import jax

                jax.devices()
                if device_ids:
                    ids = (ctypes.c_int64 * len(device_ids))(*device_ids)
                    rc = lib.axon_start_nrt_profile(ids, len(device_ids))
                else:
                    rc = lib.axon_start_nrt_profile(None, 0)
                if rc != 0:
                    raise RuntimeError(f"axon_start_nrt_profile rc={rc}")
                try:
                    yield
                finally:
                    n = lib.axon_stop_nrt_profile(str(output_dir).encode())
                    print(f"profile: {n} file(s) written to {output_dir}")

            hook = _hook
    except OSError:
        pass

    m = types.ModuleType("antenv.axon_hooks")
    m.get_axon_ntff_profile_hook = lambda: hook
    m.set_axon_ntff_profile_hook = lambda h: None
    sys.modules["antenv.axon_hooks"] = m
    antenv.axon_hooks = m


def _maybe_patch_ldw():
    """Flip walrus --enable-ldw-opt to true (LDWOPT=0 disables)."""
    import os
    import concourse.bass_utils as bu

    if os.environ.get("LDWOPT", "0") != "1" or getattr(bu, "_ldw_patched", False):
        return
    orig = bu.run_command

    def patched(argv, **kwargs):
        argv = [
            a.replace("--enable-ldw-opt=false", "--enable-ldw-opt=true")
            if isinstance(a, str)
            else a
            for a in argv
        ]
        return orig(argv, **kwargs)

    bu.run_command = patched
    bu._ldw_patched = True


def _run(inputs, trace=False):
    import concourse.bass_utils as bu
    from concourse.bass_utils import run_bass_kernel_spmd

    _maybe_patch_ldw()
    if trace:
        _ensure_ntff_hook()
        bu.upload_artifacts = lambda tmpdir: tmpdir

    in_maps, ab0, ab1 = _prepare(inputs)
    key = (ab0, ab1)
    if key not in _CACHE:
        _CACHE[key] = _build(ab0, ab1)
    nc = _CACHE[key]
    bkr = run_bass_kernel_spmd(nc, in_maps, list(range(B)), trace=trace)
    out = np.concatenate([np.asarray(bkr.results[b]["out"]) for b in range(B)], axis=0)
    return out.astype(np.float32), bkr


def kernel(**inputs) -> np.ndarray:
    return _run(inputs, trace=False)[0]


# revision 61
# speedup vs baseline: 1.0093x; 1.0011x over previous
"""Trainium2 Bass kernel for AGATCellWithMLP (gnn_message_passing).

Data-parallel across 8 NeuronCores: core b owns graph b (B=8).  Hypernetwork
weights are replicated and streamed; no collectives.

Math (per core, NOBS=512 selected rows n, CD=513 features, IS=256 out):
  combined = [x | h]                                  [1024, 513]
  scoresT[j, n] = leaky(src[n] + tgt[j] + ab)          per head
  attnT[j, n]  = exp(scoresT) * (0.5 / colsum)        (softmax + head-mean)
  selT[d, n]   = sum_h (combined^T @ attnT_h)         [513, 512]
  g(W, inT)[o, n] = sum_{d,i} qT[d,n] * inT[i,n] * W[d,i,o] + (b^T q)[o,n]
  r = sigmoid(g(Wr, selT)); u = sigmoid(g(Wu, selT))
  cT = [x_selT | r*h_selT]
  cand = tanh(g(Wc, cT))
  out[o, n] = (1-u)*r*h_selT + u*cand   -> transpose -> [512, 256]

Performance structure (from trace analysis):
 - The PE runs matmuls back-to-back at ~217ns/MM ([128c,128o,512n], LDWEIGHTS
   hidden) when its queue has no unsatisfied waits; the kernel is built so
   every stage keeps producers >=1 iteration ahead of the PE.
 - Attention is computed almost entirely on the PE: C^T via transposes
   (reused later as xs/hs), src/tgt as a [128d x 4] matmul over C^T, and
   scoresT[j,n] = src[n] + tgt[j] as a rank-3 matmul ([tgt_hi; tgt_lo; ones]
   x [ones; ones; src]) so the DVE only does the leaky-relu.
 - r/u contraction runs in fp8 e4m3 DoubleRow.  The per-d z = selT * q_d
   fp8 production (the old bottleneck: fp8 stores cost ~2.4ns/elem on DVE)
   is split: DVE writes chunks 0-1 fp8-direct (merged op ~1.4ns/elem), and
   produces chunks 2-3 in bf16 (~0.73ns/elem) which the otherwise-idle
   scalar engine converts to fp8.
 - q_d broadcast rows are DMA'd in batches of 8 d's (one descriptor set per
   batch instead of per d).

Precision: host pre-scales qT by SQ=8 and the r/u weight set by SW=64 to
center fp8 e4m3 dynamic range; the device unscales inside the sigmoid.  The
candidate gate stays bf16 (tanh preact std ~13; fp8 fails the 2e-2 gate).
"""

import numpy as np
import ml_dtypes

BF16 = ml_dtypes.bfloat16
F8 = ml_dtypes.float8_e4m3

B, N, IS, QD, H = 8, 1024, 256, 128, 2
CD = 2 * IS + 1  # 513
NOBS = N // 2  # 512
K = B * NOBS  # 4096
P = 128
NJ = N // P  # 8 j-chunks
NDC = 4  # full d-chunks of the 513 contraction (tail row handled separately)

SQ = 8.0  # host scale on qT (folded out in the gate activations)
SW = 64.0  # host scale on the r/u weight set (fp8 dynamic-range centering)

# shifted column chunks of `combined` used for C^T / xs / hs (tail col 256
# handled separately so x rows [0:256) and h rows [257:513) stay aligned)
CT_COLS = [(0, 128), (128, 256), (257, 385), (385, 513)]

_CACHE = {}


def _split_waits(nc, mybir):
    # This toolchain's walrus allows only ONE sync-wait per instruction
    # ("Too many sync wait commands"); hoist extra waits onto standalone
    # same-engine EventSemaphore nops placed immediately before.
    n = 0
    for f in nc.m.functions:
        for blk in f.blocks:
            out = []
            for inst in blk.instructions:
                si = inst.sync_info
                if si is not None and si.on_wait and len(si.on_wait) > 1:
                    waits = list(si.on_wait)
                    for w in waits[:-1]:
                        ev = mybir.InstEventSemaphore(
                            name=f"I-wsplit-{n}", engine=inst.engine, ins=[], outs=[]
                        )
                        ev.sync_info = mybir.SyncInfo(on_wait=[w], on_update=[])
                        out.append(ev)
                        n += 1
                    inst.sync_info = mybir.SyncInfo(
                        on_wait=[waits[-1]], on_update=list(si.on_update or [])
                    )
                out.append(inst)
            blk.instructions = out


def _build(ab0: float, ab1: float):
    import concourse.bass as bass
    import concourse.mybir as mybir
    import concourse.tile as tile
    from concourse.masks import make_identity
    from contextlib import ExitStack

    dt = mybir.dt
    AF = mybir.ActivationFunctionType
    ALU = mybir.AluOpType
    DR = mybir.MatmulPerfMode.DoubleRow

    nc = bass.Bass()
    # The kernel-tail EVENT_SEMAPHORE_RANGE_CLEAR (InstISA) is rejected by this
    # toolchain's walrus ("ISA wrong length"); the NEFF runs once per load, so
    # skipping the tail semaphore clear is safe.
    nc.clear_and_free_semaphores = lambda sems: None

    cmb_d = nc.declare_dram_parameter("cmb", [N, CD], dt.bfloat16, isOutput=False)
    qT_d = nc.declare_dram_parameter("qT", [QD, NOBS], dt.bfloat16, isOutput=False)
    w4t_d = nc.declare_dram_parameter("w4t", [P, NDC, 4], dt.bfloat16, isOutput=False)
    w4tail_d = nc.declare_dram_parameter("w4tail", [1, 4], dt.bfloat16, isOutput=False)
    wru_d = nc.declare_dram_parameter(
        "wru", [QD, P, NDC, 2, IS], dt.float8e4, isOutput=False
    )
    wrut_d = nc.declare_dram_parameter("wrut", [QD, 2, IS], dt.bfloat16, isOutput=False)
    wc_d = nc.declare_dram_parameter("wc", [QD, P, NDC, IS], dt.bfloat16, isOutput=False)
    wct_d = nc.declare_dram_parameter("wct", [QD, IS], dt.bfloat16, isOutput=False)
    bru_d = nc.declare_dram_parameter("bru", [QD, 2, IS], dt.bfloat16, isOutput=False)
    bc_d = nc.declare_dram_parameter("bc", [QD, IS], dt.bfloat16, isOutput=False)
    abv_d = nc.declare_dram_parameter("abv", [4, 1], dt.float32, isOutput=False)
    out_d = nc.declare_dram_parameter("out", [NOBS, IS], dt.float32, isOutput=True)
    # DRAM bounce buffers (partition remaps / broadcasts)
    sttail_d = nc.dram_tensor("sttail_dram", [1, NOBS], dt.bfloat16)
    xtail_d = nc.dram_tensor("xtail_dram", [1, NOBS], dt.bfloat16)
    tgthi_d = nc.dram_tensor("tgthi_dram", [H, N], dt.bfloat16)
    tgtlo_d = nc.dram_tensor("tgtlo_dram", [H, N], dt.bfloat16)
    src_d = nc.dram_tensor("src_dram", [H, NOBS], dt.bfloat16)

    with tile.TileContext(nc) as tc, ExitStack() as ctx:
        consts = ctx.enter_context(tc.tile_pool(name="consts", bufs=1))
        sb = ctx.enter_context(tc.tile_pool(name="sb", bufs=1))
        scratch = ctx.enter_context(tc.tile_pool(name="scratch", bufs=2))
        qbpool = ctx.enter_context(tc.tile_pool(name="qbpool", bufs=3))
        e2pool = ctx.enter_context(tc.tile_pool(name="e2pool", bufs=3))
        zpool = ctx.enter_context(tc.tile_pool(name="zpool", bufs=2))
        z4pool = ctx.enter_context(tc.tile_pool(name="z4pool", bufs=3))
        zbpool = ctx.enter_context(tc.tile_pool(name="zbpool", bufs=2))
        wpool = ctx.enter_context(tc.tile_pool(name="wpool", bufs=6))
        wcpool = ctx.enter_context(tc.tile_pool(name="wcpool", bufs=6))
        wcxpool = ctx.enter_context(tc.tile_pool(name="wcxpool", bufs=4))

        # ---------------- constants ----------------
        id_bf = consts.tile([P, P], dt.bfloat16)
        make_identity(nc, id_bf)
        id_f32 = consts.tile([P, P], dt.float32)
        make_identity(nc, id_f32)
        ones_col = consts.tile([P, 1], dt.bfloat16)
        nc.gpsimd.memset(ones_col, 1.0)
        ones_row = consts.tile([1, P], dt.bfloat16)
        nc.gpsimd.memset(ones_row, 1.0)

        # ---------------- inputs to SBUF ----------------
        C = sb.tile([P, NJ, CD], dt.bfloat16)  # combined, j-chunked
        # per-jc DMAs so the first transposes start as soon as chunk 0 lands
        for jc in range(NJ):
            nc.sync.dma_start(C[:, jc, :], cmb_d[jc * P : (jc + 1) * P, :])
        qT = sb.tile([P, NOBS], dt.bfloat16)
        nc.sync.dma_start(qT[:], qT_d[:])
        w4t = sb.tile([P, NDC, 4], dt.bfloat16)
        nc.sync.dma_start(w4t[:], w4t_d[:])
        w4tail = sb.tile([1, 4], dt.bfloat16)
        nc.sync.dma_start(w4tail[:], w4tail_d[:])
        wrut = sb.tile([P, 2, IS], dt.bfloat16)
        nc.sync.dma_start(wrut[:], wrut_d[:])
        wct = sb.tile([P, IS], dt.bfloat16)
        nc.sync.dma_start(wct[:], wct_d[:])
        bru = sb.tile([P, 2, IS], dt.bfloat16)
        nc.sync.dma_start(bru[:], bru_d[:])
        bc = sb.tile([P, IS], dt.bfloat16)
        nc.sync.dma_start(bc[:], bc_d[:])
        abv = sb.tile([4, 1], dt.float32)
        nc.sync.dma_start(abv[:], abv_d[:])

        # ---------------- c-gate accumulators (opened early: the x-part
        # matmuls are independent of attention and prepaid into its PE idle)
        psE_cm = tc.tile_pool(name="psE", bufs=1, space="PSUM")
        psE = psE_cm.__enter__()
        cacc = [
            psE.tile([P, NOBS], dt.float32, tag=f"cacc{oc}", name=f"cacc{oc}")
            for oc in range(2)
        ]
        for oc in range(2):
            nc.tensor.matmul(
                cacc[oc][:], bc[:, oc * P : (oc + 1) * P], qT[:], start=True, stop=False
            )

        def qb_batch(t8):
            qbt = qbpool.tile([P, 8, NOBS], dt.bfloat16, tag="qb8")
            nc.sync.dma_start(
                qbt[:],
                qT_d[t8 * 8 : t8 * 8 + 8, :]
                .rearrange("(o a) b -> o a b", o=1)
                .to_broadcast([P, 8, NOBS]),
            )
            return qbt

        # ---------------- C^T (all j), tails ----------------
        CT = sb.tile([P, NDC, NJ, P], dt.bfloat16)  # shifted chunks, see CT_COLS
        ctail = sb.tile([1, NJ, P], dt.bfloat16)  # combined col 256 per jc
        xsv = CT[:, 0:2, 0:4, :].rearrange("p (o a) jc b -> p o a jc b", o=1)

        PRE = 20  # c-gate x-part d-pairs prepaid during attention
        _pre = {"t": 0, "qb": None}

        def prepay_pair():
            t = _pre["t"]
            if t >= PRE:
                return
            _pre["t"] += 1
            dm = (2 * t) % 8
            if dm == 0:
                _pre["qb"] = qb_batch(t // 4)
            qbx = _pre["qb"]
            wcxs = []
            for dp in range(2):
                w = wcxpool.tile([P, 2, IS], dt.bfloat16, tag="wcx")
                nc.sync.dma_start(w[:], wc_d[2 * t + dp][:, 0:2, :])
                wcxs.append(w)
            qbp = qbx[:, dm : dm + 2, :].rearrange("p a (o b) -> p a o b", o=1)
            xq = zpool.tile([P, 2, NDC, NOBS], dt.bfloat16, tag="z2")
            nc.vector.tensor_mul(
                xq[:, :, 0:2, :],
                xsv.to_broadcast([P, 2, 2, NJ // 2, P]),
                qbp.to_broadcast([P, 2, 2, NOBS]),
            )
            for dp in range(2):
                for ic in range(2):
                    for oc in range(2):
                        nc.tensor.matmul(
                            cacc[oc][:],
                            wcxs[dp][:, ic, oc * P : (oc + 1) * P],
                            xq[:, dp, ic, :],
                            start=False,
                            stop=False,
                        )

        psT_cm = tc.tile_pool(name="psT", bufs=2, space="PSUM")
        psT = psT_cm.__enter__()

        def transpose_jc(jc):
            tp = psT.tile([P, NDC, P], dt.bfloat16, tag="tp")
            for c, (lo, hi) in enumerate(CT_COLS):
                nc.tensor.transpose(tp[:, c, :], C[:, jc, lo:hi], id_bf)
            # alternate ACT/DVE so the copy chain isn't serial on one engine
            if jc % 2 == 0:
                nc.scalar.copy(CT[:, :, jc, :], tp[:])
            else:
                nc.vector.tensor_copy(CT[:, :, jc, :], tp[:])
            tq = psT.tile([1, P], dt.bfloat16, tag="tq")
            nc.tensor.transpose(tq[:], C[:, jc, 256:257], id_bf)
            nc.scalar.copy(ctail[:, jc, :], tq[:])

        for jc in range(4):
            transpose_jc(jc)
        prepay_pair()  # xs (CT jc 0-3, chunks 0-1) is ready now
        prepay_pair()
        for jc in range(4, NJ):
            transpose_jc(jc)

        # ---------------- src/tgt via PE ----------------
        # A[k, j] for k in (tgt0, tgt1, src0, src1); two j-halves of 512
        As = sb.tile([4, 2, NOBS], dt.bfloat16)
        Alo = sb.tile([2, 2, NOBS], dt.bfloat16)
        psA_cm = tc.tile_pool(name="psA", bufs=2, space="PSUM")
        psA = psA_cm.__enter__()
        for half in range(2):
            A = psA.tile([4, NOBS], dt.float32, tag="A")
            for c in range(NDC):
                nc.tensor.matmul(
                    A[:],
                    w4t[:, c, :],
                    CT[:, c, half * 4 : half * 4 + 4, :],
                    start=(c == 0),
                    stop=False,
                )
            nc.tensor.matmul(
                A[:],
                w4tail[:],
                ctail[:, half * 4 : half * 4 + 4, :],
                start=False,
                stop=True,
            )
            # cast (+attn_b on the src rows); keep a bf16-lo residual for tgt
            nc.scalar.activation(As[:, half, :], A[:], AF.Identity, bias=abv[:])
            nc.vector.tensor_sub(Alo[:, half, :], A[0:2, :], As[0:2, half, :])

        # partition remap via DRAM bounce: TT = [tgt_hi; tgt_lo; ones],
        # RR = [ones; ones; src+ab]
        TT = sb.tile([3, H, NJ, P], dt.bfloat16)
        RR = sb.tile([3, H, NOBS], dt.bfloat16)
        # memset whole tiles (partition base must be 0); DMAs below overwrite
        # the non-ones rows
        nc.gpsimd.memset(TT[:], 1.0)
        nc.gpsimd.memset(RR[:], 1.0)
        for hh in range(H):
            for half in range(2):
                nc.sync.dma_start(
                    tgthi_d[hh : hh + 1, half * NOBS : (half + 1) * NOBS],
                    As[hh : hh + 1, half, :],
                )
                nc.sync.dma_start(
                    tgtlo_d[hh : hh + 1, half * NOBS : (half + 1) * NOBS],
                    Alo[hh : hh + 1, half, :],
                )
            nc.sync.dma_start(src_d[hh : hh + 1, :], As[2 + hh : 3 + hh, 0, :])
        for hh in range(H):
            nc.sync.dma_start(
                TT[0:1, hh, :, :], tgthi_d[hh : hh + 1, :].rearrange("o (jc p) -> o jc p", p=P)
            )
            nc.sync.dma_start(
                TT[1:2, hh, :, :], tgtlo_d[hh : hh + 1, :].rearrange("o (jc p) -> o jc p", p=P)
            )
            nc.sync.dma_start(RR[2:3, hh, :], src_d[hh : hh + 1, :])
        psA_cm.__exit__(None, None, None)
        psT_cm.__exit__(None, None, None)
        prepay_pair()
        prepay_pair()

        # ---------------- scores -> exp (+ column sums) ----------------
        expT = sb.tile([P, H, NJ, NOBS], dt.bfloat16)
        psZ_cm = tc.tile_pool(name="psZ", bufs=1, space="PSUM")
        psZ = psZ_cm.__enter__()
        psS_cm = tc.tile_pool(name="psS", bufs=4, space="PSUM")
        psS = psS_cm.__enter__()
        zt = []
        for hh in range(H):
            zth = psZ.tile([1, NOBS], dt.float32, tag=f"zt{hh}", name=f"zt{hh}")
            zt.append(zth)
            for jc in range(NJ):
                sc = psS.tile([P, NOBS], dt.float32, tag="sc")
                nc.tensor.matmul(
                    sc[:], TT[:, hh, jc, :], RR[:, hh, :], start=True, stop=True
                )
                # exp(leaky(x)) == max(exp(x), exp(0.2x)) — avoids a PSUM-read
                # TensorScalarPtr (BIR verifier rejects it) and ACT table swaps
                nc.scalar.activation(expT[:, hh, jc, :], sc[:], AF.Exp)
                e2 = e2pool.tile([P, NOBS], dt.bfloat16, tag="e2")
                nc.scalar.activation(e2[:], sc[:], AF.Exp, scale=0.2)
                nc.vector.tensor_max(expT[:, hh, jc, :], expT[:, hh, jc, :], e2[:])
                nc.tensor.matmul(
                    zth[:],
                    ones_col[:],
                    expT[:, hh, jc, :],
                    start=(jc == 0),
                    stop=(jc == NJ - 1),
                )
                prepay_pair()
        psS_cm.__exit__(None, None, None)

        # ---------------- invZ = 0.5/colsum, broadcast to [128, n] ----------
        # reciprocal on a [128, 4] transposed layout (DVE recip is per-lane
        # serial: [1,512] costs ~4us, [128,4] is ~free)
        psI_cm = tc.tile_pool(name="psI", bufs=1, space="PSUM")
        psI = psI_cm.__enter__()
        invZB = sb.tile([P, H, NOBS], dt.bfloat16)
        for hh in range(H):
            ztsb = scratch.tile([1, NOBS], dt.float32, tag="ztsb")
            nc.scalar.activation(ztsb[:], zt[hh][:], AF.Copy, scale=2.0)
            ztc = psI.tile([P, NDC], dt.float32, tag="ztc")
            for c in range(NDC):
                nc.tensor.transpose(
                    ztc[:, c : c + 1], ztsb[:, c * P : (c + 1) * P], id_f32[0:1, 0:1]
                )
            ztcs = scratch.tile([P, NDC], dt.float32, tag="ztcs")
            nc.scalar.copy(ztcs[:], ztc[:])
            izc = scratch.tile([P, NDC], dt.float32, tag="izc")
            nc.vector.reciprocal(izc[:], ztcs[:])
            izr = psI.tile([1, NOBS], dt.float32, tag="izr")
            for c in range(NDC):
                nc.tensor.transpose(izr[:, c * P : (c + 1) * P], izc[:, c : c + 1], id_f32)
            izrb = scratch.tile([1, NOBS], dt.bfloat16, tag="izrb")
            nc.scalar.copy(izrb[:], izr[:])
            ib = psI.tile([P, NOBS], dt.float32, tag="ib")
            nc.tensor.matmul(ib[:], ones_row[:], izrb[:], start=True, stop=True)
            nc.scalar.copy(invZB[:, hh, :], ib[:])
        psI_cm.__exit__(None, None, None)
        psZ_cm.__exit__(None, None, None)

        # ---------------- attnT = expT * invZB ----------------
        attnT = sb.tile([P, H, NJ, NOBS], dt.bfloat16)
        for hh in range(H):
            nc.vector.tensor_mul(
                attnT[:, hh, :, :],
                expT[:, hh, :, :],
                invZB[:, hh : hh + 1, :].to_broadcast([P, NJ, NOBS]),
            )
            prepay_pair()

        # ---------------- selT = C^T @ attnT (summed over heads) -----------
        selT = sb.tile([P, NDC, NOBS], dt.bfloat16)
        selTtail = sb.tile([1, NOBS], dt.bfloat16)
        psL_cm = tc.tile_pool(name="psL", bufs=1, space="PSUM")
        psL = psL_cm.__enter__()
        selps = []
        for dc in range(NDC):
            sp = psL.tile([P, NOBS], dt.float32, tag=f"sel{dc}", name=f"sel{dc}")
            selps.append(sp)
            first = True
            for hh in range(H):
                for jc in range(NJ):
                    nc.tensor.matmul(
                        sp[:],
                        C[:, jc, dc * P : (dc + 1) * P],
                        attnT[:, hh, jc, :],
                        start=first,
                        stop=(hh == H - 1 and jc == NJ - 1),
                    )
                    first = False
            if dc % 2 == 0:
                nc.scalar.copy(selT[:, dc, :], sp[:])
            else:
                nc.vector.tensor_copy(selT[:, dc, :], sp[:])
            prepay_pair()
        zq = psL.tile([1, NOBS], dt.float32, tag="zq", name="zq")
        first = True
        for hh in range(H):
            for jc in range(NJ):
                nc.tensor.matmul(
                    zq[:],
                    C[:, jc, 512:513],
                    attnT[:, hh, jc, :],
                    start=first,
                    stop=(hh == H - 1 and jc == NJ - 1),
                )
                first = False
        nc.scalar.copy(selTtail[:], zq[:])
        qb1_0 = qb_batch(0)  # phase-1's first broadcast batch, warmed early
        for _ in range(6):
            prepay_pair()
        psL_cm.__exit__(None, None, None)

        # ---------------- gate phase 1: r and u (fp8 DoubleRow) -------------
        psD_cm = tc.tile_pool(name="psD", bufs=1, space="PSUM")
        psD = psD_cm.__enter__()
        acc = {}
        for g in range(2):
            for oc in range(2):
                acc[(g, oc)] = psD.tile(
                    [P, NOBS], dt.float32, tag=f"acc{g}{oc}", name=f"acc{g}{oc}"
                )
        for g in range(2):
            for oc in range(2):
                nc.tensor.matmul(
                    acc[(g, oc)][:],
                    bru[:, g, oc * P : (oc + 1) * P],
                    qT[:],
                    start=True,
                    stop=False,
                )
        nc.sync.dma_start(sttail_d[:], selTtail[:])
        # tail operand prepared up front so the group-closing matmuls after
        # the d-loop never wait on the DRAM bounce roundtrip
        tb = zbpool.tile([P, NOBS], dt.bfloat16, tag="tb")
        nc.sync.dma_start(tb[:], sttail_d[0:1, :].to_broadcast([P, NOBS]))
        ztail = zbpool.tile([P, NOBS], dt.bfloat16, tag="ztail")
        nc.vector.tensor_mul(ztail[:], qT[:], tb[:])

        # d-pair batched z production: one DVE op covers two d's (halves the
        # per-op overhead and semaphore traffic on the phase-1 critical path)
        sel0 = selT[:, 0:1, :].rearrange("p (o a) b -> p o a b", o=1)
        sel13 = selT[:, 1:4, :].rearrange("p (o a) b -> p o a b", o=1)
        qb = None
        for t in range(QD // 2):
            dm = (2 * t) % 8
            if dm == 0:
                qb = qb1_0 if t == 0 else qb_batch(t // 4)
            wsls = []
            for dp in range(2):
                wsl = wpool.tile([P, NDC, 2, IS], dt.float8e4, tag="wsl")
                nc.sync.dma_start(wsl[:], wru_d[2 * t + dp])
                wsls.append(wsl)
            qbp = qb[:, dm : dm + 2, :].rearrange("p a (o b) -> p a o b", o=1)
            z4 = z4pool.tile([P, 2, NDC, NOBS], dt.float8e4, tag="z4")
            # chunk 0: DVE fp8-direct; chunks 1-3: DVE bf16 -> ACT fp8
            # (fp8 stores cost ~1.1ns/elem on DVE vs 0.63 bf16 + 0.98 ACT)
            nc.vector.tensor_mul(
                z4[:, :, 0:1, :],
                sel0.to_broadcast([P, 2, 1, NOBS]),
                qbp.to_broadcast([P, 2, 1, NOBS]),
            )
            zb = zbpool.tile([P, 2, 3, NOBS], dt.bfloat16, tag="zb")
            nc.vector.tensor_mul(
                zb[:],
                sel13.to_broadcast([P, 2, 3, NOBS]),
                qbp.to_broadcast([P, 2, 3, NOBS]),
            )
            nc.scalar.copy(z4[:, :, 1:4, :], zb[:])
            # pair-major order: all icp0 MMs (ready right after the DVE
            # fp8-direct op) run while ACT still converts chunks 2-3
            for pair in range(2):
                for dp in range(2):
                    for g in range(2):
                        for oc in range(2):
                            nc.tensor.matmul(
                                acc[(g, oc)][:],
                                wsls[dp][:, 2 * pair : 2 * pair + 2, g, oc * P : (oc + 1) * P],
                                z4[:, dp, 2 * pair : 2 * pair + 2, :],
                                start=False,
                                stop=False,
                                perf_mode=DR,
                            )
        # prefetch phase-2's first qb batch (first full pair is t=PRE) so the
        # phase boundary doesn't stall on a cold 1MB broadcast DMA
        qb_p2 = qb_batch(PRE // 4)
        # tail (i = 512): Ztail = qT * bcast(selTtail); closes the groups
        for g in range(2):
            for oc in range(2):
                nc.tensor.matmul(
                    acc[(g, oc)][:],
                    wrut[:, g, oc * P : (oc + 1) * P],
                    ztail[:],
                    start=False,
                    stop=True,
                )
        rT = sb.tile([P, 2, NOBS], dt.bfloat16)
        uT = sb.tile([P, 2, NOBS], dt.bfloat16)
        for oc in range(2):
            nc.scalar.activation(
                rT[:, oc, :], acc[(0, oc)][:], AF.Sigmoid, scale=1.0 / (SQ * SW)
            )
            nc.scalar.activation(
                uT[:, oc, :], acc[(1, oc)][:], AF.Sigmoid, scale=1.0 / (SQ * SW)
            )

        # preload the Tanh ACT table now (otherwise the output drain pays the
        # ~1.5us table swap after the sigmoids)
        warm = scratch.tile([1, 8], dt.float32, tag="warm")
        nc.scalar.activation(warm[:], qT[0:1, 0:8], AF.Tanh)
        # hc = r * h_selT  (hs = CT chunks 2-3, n < 512)
        hc = sb.tile([P, 2, NOBS], dt.bfloat16)
        nc.vector.tensor_mul(hc[:], rT[:], CT[:, 2:4, 0:4, :])
        # precompute w = (1-u)*hc now (DVE is light here) so the output tail
        # is only tanh -> mul -> add
        um = sb.tile([P, 2, NOBS], dt.bfloat16)
        nc.vector.tensor_scalar(um[:], uT[:], -1.0, 1.0, op0=ALU.mult, op1=ALU.add)
        w_uh = sb.tile([P, 2, NOBS], dt.bfloat16)
        nc.vector.tensor_mul(w_uh[:], um[:], hc[:])

        # ---------------- gate phase 2: candidate (bf16) --------------------
        # x-part for the first PRE d-pairs was prepaid during attention
        psF_cm = tc.tile_pool(name="psF", bufs=2, space="PSUM")
        psF = psF_cm.__enter__()
        nc.sync.dma_start(xtail_d[:], ctail[:, 0:4, :])
        ctb = zbpool.tile([P, NOBS], dt.bfloat16, tag="ctb")
        nc.sync.dma_start(ctb[:], xtail_d[0:1, :].to_broadcast([P, NOBS]))
        zctail = zbpool.tile([P, NOBS], dt.bfloat16, tag="zctail")
        nc.vector.tensor_mul(zctail[:], qT[:], ctb[:])
        hcv = hc[:].rearrange("p (o a) b -> p o a b", o=1)
        qb = None
        # full (x+h) pairs first: their x-part z ops don't depend on hc, so
        # the PE never stalls on the sigmoid->hc chain at the phase boundary.
        # PRE*2 is 8-aligned so qb batch boundaries line up in both regions.
        assert (2 * PRE) % 8 == 0
        for t in list(range(PRE, QD // 2)) + list(range(PRE)):
            dm = (2 * t) % 8
            if dm == 0:
                qb = qb_p2 if t == PRE else qb_batch(t // 4)
            qbp = qb[:, dm : dm + 2, :].rearrange("p a (o b) -> p a o b", o=1)
            qbb = qbp.to_broadcast([P, 2, 2, NOBS])
            z2 = zpool.tile([P, 2, NDC, NOBS], dt.bfloat16, tag="z2")
            if t < PRE:
                wsls = []
                for dp in range(2):
                    wsl = wcpool.tile([P, NDC, IS], dt.bfloat16, tag="wcsl")
                    nc.sync.dma_start(wsl[:, 2:4, :], wc_d[2 * t + dp][:, 2:4, :])
                    wsls.append(wsl)
                nc.vector.tensor_mul(
                    z2[:, :, 2:4, :], hcv.to_broadcast([P, 2, 2, NOBS]), qbb
                )
                ics = (2, 3)
            else:
                wsls = []
                for dp in range(2):
                    wsl = wcpool.tile([P, NDC, IS], dt.bfloat16, tag="wcsl")
                    nc.sync.dma_start(wsl[:], wc_d[2 * t + dp])
                    wsls.append(wsl)
                nc.vector.tensor_mul(
                    z2[:, :, 0:2, :], xsv.to_broadcast([P, 2, 2, NJ // 2, P]), qbb
                )
                nc.vector.tensor_mul(
                    z2[:, :, 2:4, :], hcv.to_broadcast([P, 2, 2, NOBS]), qbb
                )
                ics = (0, 1, 2, 3)
            for dp in range(2):
                for ic in ics:
                    for oc in range(2):
                        nc.tensor.matmul(
                            cacc[oc][:],
                            wsls[dp][:, ic, oc * P : (oc + 1) * P],
                            z2[:, dp, ic, :],
                            start=False,
                            stop=False,
                        )
        for oc in range(2):
            nc.tensor.matmul(
                cacc[oc][:],
                wct[:, oc * P : (oc + 1) * P],
                zctail[:],
                start=False,
                stop=True,
            )

        # ---------------- combine + output ----------------
        # out = hc + u * (tanh(cacc) - hc); per-oc so oc0's combine and
        # transposes overlap oc1's tail matmul + tanh
        outf = sb.tile([P, 2, NOBS], dt.float32)
        outT = sb.tile([P, NDC, IS], dt.float32)
        for oc in range(2):
            cand = scratch.tile([P, NOBS], dt.float32, tag="cand")
            nc.scalar.activation(cand[:], cacc[oc][:], AF.Tanh, scale=1.0 / SQ)
            # combine per n-half so the first transposes start while the DVE
            # finishes the second half
            for nh in range(2):
                hsl = slice(nh * IS, (nh + 1) * IS)
                ud = scratch.tile([P, IS], dt.float32, tag="ud")
                nc.vector.tensor_mul(ud[:], cand[:, hsl], uT[:, oc, hsl])
                nc.vector.tensor_add(outf[:, oc, hsl], ud[:], w_uh[:, oc, hsl])
                for ncj in (2 * nh, 2 * nh + 1):
                    pt = psF.tile([P, P], dt.float32, tag="otr")
                    nc.tensor.transpose(
                        pt[:], outf[:, oc, ncj * P : (ncj + 1) * P], id_f32
                    )
                    nc.scalar.copy(outT[:, ncj, oc * P : (oc + 1) * P], pt[:])
                    nc.sync.dma_start(
                        out_d[ncj * P : (ncj + 1) * P, oc * P : (oc + 1) * P],
                        outT[:, ncj, oc * P : (oc + 1) * P],
                    )

        psF_cm.__exit__(None, None, None)
        psD_cm.__exit__(None, None, None)
        psE_cm.__exit__(None, None, None)

    _split_waits(nc, mybir)
    return nc


def _prepare(inputs):
    x = np.asarray(inputs["x"], np.float32)
    h = np.asarray(inputs["h"], np.float32)
    q = np.asarray(inputs["query_vectors"], np.float32)
    attn_w = np.asarray(inputs["attn_w"], np.float32)
    attn_b = np.asarray(inputs["attn_b"], np.float32)
    Wr = np.asarray(inputs["Wr"], np.float32)
    br = np.asarray(inputs["br"], np.float32)
    Wu = np.asarray(inputs["Wu"], np.float32)
    bu = np.asarray(inputs["bu"], np.float32)
    Wc = np.asarray(inputs["Wc"], np.float32)
    bc_ = np.asarray(inputs["bc"], np.float32)
    b_idx = np.asarray(inputs["b_idx"])
    n_idx = np.asarray(inputs["n_idx"])

    assert np.array_equal(b_idx, np.repeat(np.arange(B), NOBS)), "b_idx pattern"
    assert np.array_equal(n_idx, np.tile(np.arange(NOBS), B)), "n_idx pattern"

    cmb = np.concatenate([x, h], axis=-1).astype(BF16)  # [B, N, CD]

    def retile_main(W, dtype):
        # [128, 513, 256] -> rows r of the 512-row main block -> [d, i_lo, ic, o]
        m = W.astype(dtype)
        return m.reshape(QD, NDC, P, IS).transpose(0, 2, 1, 3)

    wr_m = retile_main(Wr[:, :512, :] * SW, F8)
    wu_m = retile_main(Wu[:, :512, :] * SW, F8)
    wru = np.ascontiguousarray(np.stack([wr_m, wu_m], axis=3))  # [d, i_lo, ic, g, o]
    wrut = np.ascontiguousarray(
        (np.stack([Wr[:, 512, :], Wu[:, 512, :]], axis=1) * SW).astype(BF16)
    )
    c_rows = np.r_[0:256, 257:513]
    wc = np.ascontiguousarray(retile_main(Wc[:, c_rows, :], BF16))
    wct = np.ascontiguousarray(Wc[:, 256, :].astype(BF16))
    bru = np.ascontiguousarray((np.stack([br, bu], axis=1) * SW).astype(BF16))
    bcb = np.ascontiguousarray(bc_.astype(BF16))

    # attention weights in C^T-chunk layout: w4t[p, c, k] with k order
    # (tgt0, tgt1, src0, src1); tail = combined col 256
    w_src = attn_w[:, :CD]  # [2, 513]
    w_tgt = attn_w[:, CD:]
    w4 = np.stack([w_tgt[0], w_tgt[1], w_src[0], w_src[1]], axis=1)  # [513, 4]
    w4t = np.zeros((P, NDC, 4), np.float32)
    for c, (lo, hi) in enumerate(CT_COLS):
        w4t[:, c, :] = w4[lo:hi, :]
    w4t = np.ascontiguousarray(w4t.astype(BF16))
    w4tail = np.ascontiguousarray(w4[256:257, :].astype(BF16))
    abv = np.array([[0.0], [0.0], [attn_b[0]], [attn_b[1]]], np.float32)

    in_maps = []
    for b in range(B):
        qTb = np.ascontiguousarray((q[b * NOBS : (b + 1) * NOBS].T * SQ).astype(BF16))
        in_maps.append(
            {
                "cmb": np.ascontiguousarray(cmb[b]),
                "qT": qTb,
                "w4t": w4t,
                "w4tail": w4tail,
                "wru": wru,
                "wrut": wrut,
                "wc": wc,
                "wct": wct,
                "bru": bru,
                "bc": bcb,
                "abv": abv,
            }
        )
    return in_maps, float(attn_b[0]), float(attn_b[1])


def _ensure_ntff_hook():
    """Provide antenv.axon_hooks (missing in this image) so trace=True works."""
    import sys, types, contextlib, ctypes

    try:
        import antenv.axon_hooks  # noqa: F401

        return
    except ImportError:
        pass
    import antenv

    so_path = "/opt/axon/libaxon_pjrt.so"
    hook = None
    try:
        lib = ctypes.CDLL(so_path)
        if hasattr(lib, "axon_start_nrt_profile"):
            lib.axon_start_nrt_profile.argtypes = [
                ctypes.POINTER(ctypes.c_int64),
                ctypes.c_size_t,
            ]
            lib.axon_start_nrt_profile.restype = ctypes.c_int64
            lib.axon_stop_nrt_profile.argtypes = [ctypes.c_char_p]
            lib.axon_stop_nrt_profile.restype = ctypes.c_int64

            @contextlib.contextmanager
            def _hook(output_dir, device_ids):
                import jax

                jax.devices()
                if device_ids:
                    ids = (ctypes.c_int64 * len(device_ids))(*device_ids)
                    rc = lib.axon_start_nrt_profile(ids, len(device_ids))
                else:
                    rc = lib.axon_start_nrt_profile(None, 0)
                if rc != 0:
                    raise RuntimeError(f"axon_start_nrt_profile rc={rc}")
                try:
                    yield
                finally:
                    n = lib.axon_stop_nrt_profile(str(output_dir).encode())
                    print(f"profile: {n} file(s) written to {output_dir}")

            hook = _hook
    except OSError:
        pass

    m = types.ModuleType("antenv.axon_hooks")
    m.get_axon_ntff_profile_hook = lambda: hook
    m.set_axon_ntff_profile_hook = lambda h: None
    sys.modules["antenv.axon_hooks"] = m
    antenv.axon_hooks = m


def _maybe_patch_ldw():
    """Flip walrus --enable-ldw-opt to true (LDWOPT=0 disables)."""
    import os
    import concourse.bass_utils as bu

    if os.environ.get("LDWOPT", "0") != "1" or getattr(bu, "_ldw_patched", False):
        return
    orig = bu.run_command

    def patched(argv, **kwargs):
        argv = [
            a.replace("--enable-ldw-opt=false", "--enable-ldw-opt=true")
            if isinstance(a, str)
            else a
            for a in argv
        ]
        return orig(argv, **kwargs)

    bu.run_command = patched
    bu._ldw_patched = True


def _run(inputs, trace=False):
    import concourse.bass_utils as bu
    from concourse.bass_utils import run_bass_kernel_spmd

    _maybe_patch_ldw()
    if trace:
        _ensure_ntff_hook()
        bu.upload_artifacts = lambda tmpdir: tmpdir

    in_maps, ab0, ab1 = _prepare(inputs)
    key = (ab0, ab1)
    if key not in _CACHE:
        _CACHE[key] = _build(ab0, ab1)
    nc = _CACHE[key]
    bkr = run_bass_kernel_spmd(nc, in_maps, list(range(B)), trace=trace)
    out = np.concatenate([np.asarray(bkr.results[b]["out"]) for b in range(B)], axis=0)
    return out.astype(np.float32), bkr


def kernel(**inputs) -> np.ndarray:
    return _run(inputs, trace=False)[0]
